# revision 1
# baseline (speedup 1.0000x reference)
"""Trainium2 Bass kernel: ViT-style transformer block with polynomial attention.

Sharding: pure data-parallel over batch B=32 across 8 NeuronCores (4 batch
elements per core).  No collectives.  Each core computes the full block for
its batch slice; host gathers/concats.

Per-core layout strategy:
  - tokens padded per-batch 577 -> 640 (5 tiles of 128); 4*640 = 2560 tokens
  - LayerNorm / per-token ops run token-major ([tok128, feat]); matmul
    contractions need feature-major ([feat, tok]) operands, produced with PE
    transposes.  LN gains/biases are folded into the following matmul weights
    on the host, so on-chip LN is just center+scale.
  - all matmul operands are bf16 (1 cyc/row at any free size + fast weight
    load); accumulation stays fp32 in PSUM.  LN stats / residual stream are
    fp32.
  - polynomial attention: scores_T computed per (batch, head) as [m, n] with
    K=64; the two heads of a pair are interleaved so their K=64 matmuls sit
    in different PE row-groups and overlap.  poly = relu(A r^2 + B r + C) via
    ACT Square + DVE add/max (A>0 fast path folds A into the normalizer eps);
    attn@v uses a masked-ones column appended to v so Z = sum_m(valid) poly
    lands in the same PSUM tile; normalization happens token-major via
    per-partition reciprocal scale.  Scores/poly only computed for the 578
    (=NV) columns that cover valid query tokens.
  - PolyGELU is a pure Square on ACT: the quadratic's scale folds into the
    fc2 weights and the constant folds into the fc2 bias (via column sums).
"""

import sys

for _p in ("/opt/trn_rl_repo",):
    if _p not in sys.path:
        sys.path.insert(0, _p)

from contextlib import ExitStack

import numpy as np
import ml_dtypes

import concourse.bacc as bacc
import concourse.mybir as mybir
import concourse.tile as tile

B, N, D, H = 32, 577, 384, 6
HD = D // H            # 64
HID = 4 * D            # 1536
LN_EPS = 1e-5
ATTN_EPS = 1e-6

NCORES = 8
BPC = B // NCORES      # 4 batches per core
NP = 640               # padded tokens per batch (5 * 128)
NT = NP // 128         # 5 token tiles per batch
TP = BPC * NP          # 2560 tokens per core
GT = TP // 128         # 20 token tiles per core
KC_D = D // 128        # 3 contraction chunks over D
FC_H = HID // 128      # 12 chunks over hidden
NV = N + 1             # 578: even score/poly width covering valid n tokens

F32 = mybir.dt.float32
BF16 = mybir.dt.bfloat16
AF = mybir.ActivationFunctionType
ALU = mybir.AluOpType

MT = BF16              # matmul operand dtype
AVT = BF16             # poly / v dtype
NPBF = np.dtype(ml_dtypes.bfloat16)

import os as _os
LVL = int(_os.environ.get("K_LVL", "99"))
SUB = int(_os.environ.get("K_SUB", "99"))


def _ln(nc, pools, x_t, out_t):
    """LayerNorm center+scale (gain/bias folded into downstream weights).
    Mean/var in one DVE pass (bn_stats+bn_aggr); h = x*rstd - mu*rstd.
    x_t: [128, D] sbuf tile; out_t: [128, D] (may be bf16)."""
    st = pools["st"]
    s6 = st.tile([128, 6], F32, tag="s6", name="s6")
    nc.vector.bn_stats(s6, x_t)
    mv = st.tile([128, 2], F32, tag="mv", name="mv")
    nc.vector.bn_aggr(mv, s6)
    sd = st.tile([128, 1], F32, tag="sd", name="sd")
    nc.scalar.activation(sd, mv[:, 1:2], AF.Sqrt, bias=pools["lneps"])
    rstd = st.tile([128, 1], F32, tag="rstd", name="rstd")
    nc.vector.reciprocal(rstd, sd)
    mr = st.tile([128, 1], F32, tag="mr", name="mr")
    nc.gpsimd.tensor_mul(mr, mv[:, 0:1], rstd)
    nc.vector.tensor_scalar(out=out_t, in0=x_t, scalar1=rstd, scalar2=mr,
                            op0=ALU.mult, op1=ALU.subtract)


def _transpose_128(nc, pools, src_t, dst_t, dst_col, ident, copy_engine):
    """PE-transpose a [128,128] block of src into dst[:, dst_col:dst_col+128]."""
    tp = pools["pp"].tile([128, 128], src_t.dtype, tag="mm", name="tp_ps",
                          space="PSUM", bufs=pools["mmbufs"])
    nc.tensor.transpose(tp, src_t, ident)
    if copy_engine == "act":
        nc.scalar.activation(dst_t[:, dst_col:dst_col + 128], tp, AF.Copy)
    elif copy_engine == "gpsimd":
        nc.gpsimd.tensor_copy(dst_t[:, dst_col:dst_col + 128], tp)
    else:
        nc.vector.tensor_copy(dst_t[:, dst_col:dst_col + 128], tp)


def build_program(sc, bench_R=0):
    """sc: dict of host scalar constants / flags."""
    nc = bacc.Bacc("TRN2", target_bir_lowering=False, debug=False)

    kind_in = "Internal" if bench_R else "ExternalInput"
    xp = nc.dram_tensor("xp", [TP, D], F32, kind=kind_in).ap()
    wqkv_d = nc.dram_tensor("wqkv", [128, KC_D * 3 * D], MT, kind=kind_in).ap()
    wproj_d = nc.dram_tensor("wproj", [128, KC_D * D], MT, kind=kind_in).ap()
    wfc1_d = nc.dram_tensor("wfc1", [128, KC_D * HID], MT, kind=kind_in).ap()
    wfc2_d = nc.dram_tensor("wfc2", [128, FC_H * D], MT, kind=kind_in).ap()
    qkb_d = nc.dram_tensor("qkb", [128, 6], F32, kind=kind_in).ap()
    vbr_d = nc.dram_tensor("vbr", [128, D], F32, kind=kind_in).ap()
    pbr_d = nc.dram_tensor("pbr", [128, D], F32, kind=kind_in).ap()
    f2br_d = nc.dram_tensor("f2br", [128, D], F32, kind=kind_in).ap()
    g1b_d = nc.dram_tensor("g1b", [128, FC_H], F32, kind=kind_in).ap()
    vone6_d = nc.dram_tensor("vone6", [128, NT * H * 2], F32, kind=kind_in).ap()
    ident_d = nc.dram_tensor("ident", [128, 128], MT, kind=kind_in).ap()
    outp = nc.dram_tensor("outp", [128 if bench_R else TP, D], F32, kind="ExternalOutput").ap()

    A2 = sc["B2A"]          # B/(2A): pass1 bias
    C2A = sc["C2A"]         # (C - B^2/(4A))/A: pass2 add before max(.,0)
    polymode = sc["polymode"]  # "fold" (A>0), "relu" (A<0), "lin" (A==0)
    pC2 = sc["pC2"]         # C - B^2/(4A) for relu mode
    pA = sc["pA"]
    pB = sc["pB"]
    pC = sc["pC"]
    gelmode = sc["gelmode"]
    add_vb = sc["add_vb"]
    add_pb = sc["add_pb"]
    add_f2b = sc["add_f2b"]

    with ExitStack() as octx:
        tc = octx.enter_context(tile.TileContext(nc))
        cp = octx.enter_context(tc.tile_pool(name="consts", bufs=1))
        # persistent constants — attention-phase tensors first so the PE can
        # start as soon as possible; MLP weights arrive much later.
        ident = cp.tile([128, 128], MT, name="ident_sb")
        nc.scalar.dma_start(ident, ident_d)
        wqkv = cp.tile([128, KC_D * 3 * D], MT, name="wqkv_sb")
        nc.scalar.dma_start(wqkv, wqkv_d)
        qkb = cp.tile([128, 6], F32, name="qkb_sb")
        nc.scalar.dma_start(qkb, qkb_d)
        vone6 = cp.tile([128, NT * H * 2], F32, name="vone6_sb")
        nc.scalar.dma_start(vone6, vone6_d)
        vbr = cp.tile([128, D], F32, name="vbr_sb")
        nc.scalar.dma_start(vbr, vbr_d)
        wproj = cp.tile([128, KC_D * D], MT, name="wproj_sb")
        nc.scalar.dma_start(wproj, wproj_d)
        pbr = cp.tile([128, D], F32, name="pbr_sb")
        nc.scalar.dma_start(pbr, pbr_d)
        wfc1 = cp.tile([128, KC_D * HID], MT, name="wfc1_sb")
        nc.scalar.dma_start(wfc1, wfc1_d)
        g1b = cp.tile([128, FC_H], F32, name="g1b_sb")
        nc.scalar.dma_start(g1b, g1b_d)
        wfc2 = cp.tile([128, FC_H * D], MT, name="wfc2_sb")
        nc.scalar.dma_start(wfc2, wfc2_d)
        f2br = cp.tile([128, D], F32, name="f2br_sb")
        nc.scalar.dma_start(f2br, f2br_d)
        c_eps = cp.tile([128, 1], F32, name="c_eps")
        nc.vector.memset(c_eps, sc["eps_eff"])
        c_lneps = cp.tile([128, 1], F32, name="c_lneps")
        nc.vector.memset(c_lneps, LN_EPS)
        c_b2a = cp.tile([128, 1], F32, name="c_b2a")
        nc.vector.memset(c_b2a, A2)
        c_pc2 = cp.tile([128, 1], F32, name="c_pc2")
        nc.vector.memset(c_pc2, pC2 if polymode == "relu" else pC)

        x2p = octx.enter_context(tc.tile_pool(name="x2p", bufs=GT))
        x2_tiles = [None] * GT

        def _body_phases():
            # ---------------- Phase A: LN1 + QKV + attention + proj ----------
            with ExitStack() as actx:
                pp = actx.enter_context(tc.tile_pool(name="ppA", bufs=1, space="PSUM"))
                lnp = {
                    "st": actx.enter_context(tc.tile_pool(name="stA", bufs=4)),
                    "pp": pp,
                    "mmbufs": 2,
                    "lneps": c_lneps,
                }
                xpool = actx.enter_context(tc.tile_pool(name="xA", bufs=NT + 1))
                hpool = actx.enter_context(tc.tile_pool(name="hA", bufs=2))
                hTp = actx.enter_context(tc.tile_pool(name="hT", bufs=6))
                qkp = actx.enter_context(tc.tile_pool(name="qk", bufs=6))
                vp = actx.enter_context(tc.tile_pool(name="vp", bufs=NT))
                polyp = actx.enter_context(tc.tile_pool(name="poly", bufs=7))
                tsbp = actx.enter_context(tc.tile_pool(name="tsb", bufs=4))
                aop = actx.enter_context(tc.tile_pool(name="ao", bufs=NT))
                aTp = actx.enter_context(tc.tile_pool(name="aT", bufs=3))

                for b in range(BPC):
                    # --- LN1 + transpose to feature-major h_T ---
                    hT = []
                    x_ts = []
                    for kc in range(KC_D):
                        t = hTp.tile([128, NP], MT, tag="hT", name=f"hT{b}_{kc}")
                        hT.append(t)
                    for tt in range(NT):
                        gt = b * NT + tt
                        x_t = xpool.tile([128, D], F32, tag="x", name="x_t")
                        x_ts.append(x_t)
                        nc.sync.dma_start(x_t, xp[gt * 128:(gt + 1) * 128, :])
                        h_t = hpool.tile([128, D], MT, tag="h", name="h_t")
                        _ln(nc, lnp, x_t, h_t)
                        for kc in range(KC_D):
                            _transpose_128(nc, lnp, h_t[:, kc * 128:(kc + 1) * 128],
                                           hT[kc], tt * 128, ident, "act")

                    # --- QKV ---
                    qk = []
                    for fc in range(6):  # q: 0..2, k: 3..5 (feature chunks of 128)
                        t = qkp.tile([128, NP], MT, tag="qk", name=f"qk{b}_{fc}")
                        qk.append(t)
                        for c0, c1 in ((0, 320), (320, 640)):
                            ps = pp.tile([128, 320], F32, tag="mm", name="qk_ps",
                                         space="PSUM", bufs=2)
                            for kc in range(KC_D):
                                nc.tensor.matmul(
                                    ps[:, 0:c1 - c0],
                                    wqkv[:, kc * 3 * D + fc * 128:
                                         kc * 3 * D + fc * 128 + 128],
                                    hT[kc][:, c0:c1],
                                    start=(kc == 0), stop=(kc == KC_D - 1),
                                )
                            nc.vector.tensor_scalar(
                                out=t[:, c0:c1], in0=ps[:, 0:c1 - c0],
                                scalar1=qkb[:, fc:fc + 1], scalar2=None,
                                op0=ALU.add)

                    v_sb = []
                    for tt in range(NT):
                        ps = pp.tile([128, D], F32, tag="mm", name="v_ps", space="PSUM",
                                     bufs=2)
                        for kc in range(KC_D):
                            nc.tensor.matmul(
                                ps,
                                hT[kc][:, tt * 128:(tt + 1) * 128],
                                wqkv[:, kc * 3 * D + 768:kc * 3 * D + 1152],
                                start=(kc == 0), stop=(kc == KC_D - 1),
                            )
                        vt = vp.tile([128, H * (HD + 2)], AVT, tag="v", name=f"v{b}_{tt}")
                        v_sb.append(vt)
                        v3 = vt.rearrange("p (h c) -> p h c", c=HD + 2)
                        ps3 = ps.rearrange("p (h c) -> p h c", c=HD)
                        if add_vb:
                            nc.vector.tensor_add(
                                v3[:, :, 0:HD], ps3,
                                vbr.rearrange("p (h c) -> p h c", c=HD))
                            if tt == NT - 1:
                                # zero the padded-token rows via the valid mask
                                nc.vector.tensor_scalar(
                                    out=v3[:, :, 0:HD], in0=v3[:, :, 0:HD],
                                    scalar1=vone6[:, tt * H * 2:tt * H * 2 + 1],
                                    scalar2=None, op0=ALU.mult)
                        else:
                            # padded-token rows of psum are exactly 0 (h_pad == 0)
                            nc.vector.tensor_copy(v3[:, :, 0:HD], ps3)
                        # masked-ones column + zero pad column per head
                        nc.vector.tensor_copy(
                            v3[:, :, HD:HD + 2],
                            vone6[:, tt * H * 2:(tt + 1) * H * 2]
                            .rearrange("p (h c) -> p h c", c=2))

                    # --- attention: head pairs, scores_T + poly (n<NV), av ---
                    ao_t = []
                    for nt in range(NT):
                        t = aop.tile([128, D], MT, tag="ao", name=f"ao{b}_{nt}")
                        ao_t.append(t)
                    for hp in range(H // 2):
                        h0, h1 = 2 * hp, 2 * hp + 1
                        polys = {}
                        for h in (h0, h1):
                            for mc in range(NT):
                                polys[(h, mc)] = polyp.tile(
                                    [128, NP], AVT, tag="poly",
                                    name=f"poly{h}_{mc}", bufs=20)
                        # two heads interleaved: their K=64 matmuls live in
                        # PE row-groups 0-63 / 64-127 and overlap.
                        for mc in range(NT):
                            tsb = {}
                            for h in (h0, h1):
                                tsb[h] = tsbp.tile([128, NP], F32, tag="tsb",
                                                   name=f"t_sb{h % 2}")
                            for c0, c1 in ((0, 320), (320, NV)):
                                pss = {}
                                for h in (h0, h1):
                                    fcq = h // 2
                                    row = (h % 2) * 64
                                    ps = pp.tile([128, 320], F32, tag="sc",
                                                 name="sc_ps", space="PSUM",
                                                 bufs=4)
                                    nc.tensor.matmul(
                                        ps[:, 0:c1 - c0],
                                        qk[3 + fcq][row:row + 64,
                                                    mc * 128:(mc + 1) * 128],
                                        qk[fcq][row:row + 64, c0:c1],
                                        start=True, stop=True,
                                    )
                                    pss[h] = ps
                                for h in (h0, h1):
                                    if polymode == "lin":
                                        nc.scalar.activation(
                                            polys[(h, mc)][:, c0:c1],
                                            pss[h][:, 0:c1 - c0], AF.Relu,
                                            scale=pB, bias=c_pc2)
                                    else:
                                        nc.scalar.activation(
                                            tsb[h][:, c0:c1],
                                            pss[h][:, 0:c1 - c0],
                                            AF.Square, bias=c_b2a)
                            for h in (h0, h1):
                                if polymode == "fold":
                                    nc.vector.tensor_scalar(
                                        out=polys[(h, mc)][:, 0:NV],
                                        in0=tsb[h][:, 0:NV], scalar1=C2A,
                                        scalar2=0.0, op0=ALU.add, op1=ALU.max)
                                elif polymode == "relu":
                                    nc.scalar.activation(
                                        polys[(h, mc)][:, 0:NV],
                                        tsb[h][:, 0:NV], AF.Relu,
                                        scale=pA, bias=c_pc2)

                        # attn @ [v | ones] for the pair; Z cols batched
                        for nt in range(NT):
                            ps = pp.tile([128, 2 * (HD + 2)], F32, tag="av",
                                         name="av_ps", space="PSUM", bufs=2)
                            for j, h in enumerate((h0, h1)):
                                off = j * (HD + 2)
                                for mc in range(NT):
                                    nc.tensor.matmul(
                                        ps[:, off:off + HD + 2],
                                        polys[(h, mc)][:,
                                                       nt * 128:(nt + 1) * 128],
                                        v_sb[mc][:, h * (HD + 2):
                                                 (h + 1) * (HD + 2)],
                                        start=(mc == 0), stop=(mc == NT - 1),
                                    )
                            z2 = ps.rearrange("p (j c) -> p j c", c=HD + 2)[:, :, HD]
                            zt = lnp["st"].tile([128, 2], F32, tag="zt", name="zt")
                            nc.vector.tensor_scalar_add(zt, z2, float(sc["eps_eff"]))
                            rz = lnp["st"].tile([128, 2], F32, tag="rz", name="rz")
                            nc.vector.reciprocal(rz, zt)
                            for j, h in enumerate((h0, h1)):
                                off = j * (HD + 2)
                                nc.vector.tensor_scalar(
                                    out=ao_t[nt][:, h * HD:(h + 1) * HD],
                                    in0=ps[:, off:off + HD],
                                    scalar1=rz[:, j:j + 1], scalar2=None,
                                    op0=ALU.mult)

                    # --- transpose attn_out, proj, residual ---
                    aT = []
                    for kc in range(KC_D):
                        t = aTp.tile([128, NP], MT, tag="aT", name=f"aT{b}_{kc}")
                        aT.append(t)
                    for nt in range(NT):
                        for kc in range(KC_D):
                            _transpose_128(nc, lnp, ao_t[nt][:, kc * 128:(kc + 1) * 128],
                                           aT[kc], nt * 128, ident, "vec")
                    for tt in range(NT):
                        gt = b * NT + tt
                        ps = pp.tile([128, D], F32, tag="mm", name="pj_ps", space="PSUM",
                                     bufs=2)
                        for kc in range(KC_D):
                            nc.tensor.matmul(
                                ps,
                                aT[kc][:, tt * 128:(tt + 1) * 128],
                                wproj[:, kc * D:(kc + 1) * D],
                                start=(kc == 0), stop=(kc == KC_D - 1),
                            )
                        x2t = x2p.tile([128, D], F32, tag="x2", name=f"x2_{gt}")
                        x2_tiles[gt] = x2t
                        if add_pb:
                            nc.vector.tensor_add(x2t, ps, pbr)
                            nc.vector.tensor_add(x2t, x2t, x_ts[tt])
                        else:
                            nc.vector.tensor_add(x2t, ps, x_ts[tt])

            # ---------------- Phase B/C: LN2 + MLP ----------------
            with ExitStack() as mctx:
                pp = mctx.enter_context(tc.tile_pool(name="ppM", bufs=1, space="PSUM"))
                lnp = {
                    "st": mctx.enter_context(tc.tile_pool(name="stM", bufs=4)),
                    "pp": pp,
                    "mmbufs": 8,
                    "lneps": c_lneps,
                }
                hpool = mctx.enter_context(tc.tile_pool(name="hM", bufs=2))
                h2Tp = mctx.enter_context(tc.tile_pool(name="h2T", bufs=KC_D))
                gp = mctx.enter_context(tc.tile_pool(name="gp", bufs=14))
                outpl = mctx.enter_context(tc.tile_pool(name="outl", bufs=3))

                h2T = []
                for kc in range(KC_D):
                    t = h2Tp.tile([128, TP], MT, tag="h2T", name=f"h2T_{kc}")
                    h2T.append(t)
                for gt in range(GT):
                    h_t = hpool.tile([128, D], MT, tag="h2", name="h2_t")
                    _ln(nc, lnp, x2_tiles[gt], h_t)
                    for kc in range(KC_D):
                        _transpose_128(nc, lnp, h_t[:, kc * 128:(kc + 1) * 128],
                                       h2T[kc], gt * 128, ident, "act")

                NCH = TP // 512  # 5 column chunks of 512
                for nch in range(NCH):
                    g_sb = []
                    for fc in range(FC_H):
                        ps = pp.tile([128, 512], F32, tag="mm", name="f1_ps",
                                     space="PSUM", bufs=8)
                        for kc in range(KC_D):
                            nc.tensor.matmul(
                                ps,
                                wfc1[:, kc * HID + fc * 128:
                                     kc * HID + fc * 128 + 128],
                                h2T[kc][:, nch * 512:(nch + 1) * 512],
                                start=(kc == 0), stop=(kc == KC_D - 1),
                            )
                        gt_sb = gp.tile([128, 512], MT, tag="g", name=f"g{nch}_{fc}")
                        g_sb.append(gt_sb)
                        # PolyGELU: quadratic scale/constant folded into
                        # fc2 weights/bias on host; here just (u+b)^2.
                        nc.scalar.activation(
                            gt_sb, ps,
                            AF.Square if gelmode == "quad" else AF.Identity,
                            bias=g1b[:, fc:fc + 1])
                    for tt in range(4):
                        gt = nch * 4 + tt
                        ps = pp.tile([128, D], F32, tag="mm", name="f2_ps",
                                     space="PSUM", bufs=8)
                        for fc in range(FC_H):
                            nc.tensor.matmul(
                                ps,
                                g_sb[fc][:, tt * 128:(tt + 1) * 128],
                                wfc2[:, fc * D:(fc + 1) * D],
                                start=(fc == 0), stop=(fc == FC_H - 1),
                            )
                        ot = outpl.tile([128, D], F32, tag="ot", name="out_t")
                        if add_f2b:
                            nc.vector.tensor_add(ot, ps, f2br)
                            nc.vector.tensor_add(ot, ot, x2_tiles[gt])
                        else:
                            nc.vector.tensor_add(ot, ps, x2_tiles[gt])
                        nc.sync.dma_start(outp[0:128, :] if bench_R else outp[gt * 128:(gt + 1) * 128, :], ot)

        if bench_R:
            with tc.For_i(0, bench_R, 1):
                _body_phases()
        else:
            _body_phases()

    nc.compile()
    return nc


def host_prep(inputs):
    """Fold LN affine params into weights, build per-core input maps."""
    f = lambda k: np.asarray(inputs[k], dtype=np.float32)
    x = f("x")
    qkv_w, qkv_b = f("qkv_w"), f("qkv_b")
    proj_w, proj_b = f("proj_w"), f("proj_b")
    fc1_w, fc1_b = f("fc1_w"), f("fc1_b")
    fc2_w, fc2_b = f("fc2_w"), f("fc2_b")
    ln1_g, ln1_b = f("ln1_g"), f("ln1_b")
    ln2_g, ln2_b = f("ln2_g"), f("ln2_b")
    attn_abc = f("attn_abc")
    gelu_abc = f("gelu_abc")

    scale = HD ** -0.5
    pA = float(attn_abc[0]) * scale * scale
    pB = float(attn_abc[1]) * scale
    pC = float(attn_abc[2])
    sc = {"pA": pA, "pB": pB, "pC": pC}
    if pA > 1e-12:
        sc["polymode"] = "fold"
        sc["B2A"] = pB / (2.0 * pA)
        sc["pC2"] = pC - pB * pB / (4.0 * pA)
        sc["C2A"] = sc["pC2"] / pA
        eps_eff = ATTN_EPS / pA
    elif pA < -1e-12:
        sc["polymode"] = "relu"
        sc["B2A"] = pB / (2.0 * pA)
        sc["pC2"] = pC - pB * pB / (4.0 * pA)
        sc["C2A"] = 0.0
        eps_eff = ATTN_EPS
    else:
        sc["polymode"] = "lin"
        sc["B2A"] = 0.0
        sc["pC2"] = pC
        sc["C2A"] = 0.0
        eps_eff = ATTN_EPS

    ga, gb, gc = float(gelu_abc[0]), float(gelu_abc[1]), float(gelu_abc[2])
    fc1_b_eff = (fc1_b + ln2_b @ fc1_w).astype(np.float32)
    if abs(ga) > 1e-12:
        # gelu(u) = ga*(u + gb/2ga)^2 + (gc - gb^2/4ga); fold ga into fc2_w
        # and the constant into fc2_b via column sums.
        sc["gelmode"] = "quad"
        gC2 = gc - gb * gb / (4.0 * ga)
        g1bias = fc1_b_eff + gb / (2.0 * ga)
        fc2_w_eff = ga * fc2_w
        fc2_b_eff = fc2_b + gC2 * fc2_w.sum(axis=0)
    else:
        # gelu(u) = gb*u + gc
        sc["gelmode"] = "lin"
        g1bias = fc1_b_eff
        fc2_w_eff = gb * fc2_w
        fc2_b_eff = fc2_b + gc * fc2_w.sum(axis=0)

    sc["eps_eff"] = float(eps_eff)
    qkv_w_eff = (ln1_g[:, None] * qkv_w).astype(np.float32)
    qkv_b_eff = (qkv_b + ln1_b @ qkv_w).astype(np.float32)
    fc1_w_eff = (ln2_g[:, None] * fc1_w).astype(np.float32)

    sc["add_vb"] = bool(np.any(qkv_b_eff[2 * D:] != 0.0))
    sc["add_pb"] = bool(np.any(proj_b != 0.0))
    sc["add_f2b"] = bool(np.any(fc2_b_eff != 0.0))

    common = {
        "wqkv": np.ascontiguousarray(
            qkv_w_eff.reshape(KC_D, 128, 3 * D).transpose(1, 0, 2)
            .reshape(128, KC_D * 3 * D)).astype(NPBF),
        "wproj": np.ascontiguousarray(
            proj_w.reshape(KC_D, 128, D).transpose(1, 0, 2)
            .reshape(128, KC_D * D)).astype(NPBF),
        "wfc1": np.ascontiguousarray(
            fc1_w_eff.reshape(KC_D, 128, HID).transpose(1, 0, 2)
            .reshape(128, KC_D * HID)).astype(NPBF),
        "wfc2": np.ascontiguousarray(
            fc2_w_eff.astype(np.float32).reshape(FC_H, 128, D).transpose(1, 0, 2)
            .reshape(128, FC_H * D)).astype(NPBF),
        "qkb": np.ascontiguousarray(qkv_b_eff[:2 * D].reshape(6, 128).T),
        "vbr": np.ascontiguousarray(
            np.broadcast_to(qkv_b_eff[2 * D:], (128, D))),
        "pbr": np.ascontiguousarray(np.broadcast_to(proj_b, (128, D))),
        "f2br": np.ascontiguousarray(np.broadcast_to(fc2_b_eff, (128, D))),
        "g1b": np.ascontiguousarray(g1bias.reshape(FC_H, 128).T),
        "vone6": None,
        "ident": np.eye(128, dtype=np.float32).astype(NPBF),
    }
    mask = (np.arange(NP) < N).astype(np.float32)  # [640]
    mz = np.zeros((NT, 128, H, 2), np.float32)
    mz[:, :, :, 0] = mask.reshape(NT, 128)[:, :, None]
    common["vone6"] = np.ascontiguousarray(
        mz.transpose(1, 0, 2, 3).reshape(128, NT * H * 2))
    common = {k: (np.ascontiguousarray(v, dtype=np.float32)
                  if v.dtype != NPBF else v)
              for k, v in common.items()}

    in_maps = []
    for c in range(NCORES):
        xp_c = np.zeros((BPC, NP, D), np.float32)
        xp_c[:, :N, :] = x[c * BPC:(c + 1) * BPC]
        m = dict(common)
        m["xp"] = xp_c.reshape(TP, D)
        in_maps.append(m)
    return sc, in_maps


_CACHE = {}


def _get_program(sc):
    key = tuple(sorted((k, v) for k, v in sc.items()))
    if key not in _CACHE:
        _CACHE[key] = build_program(sc)
    return _CACHE[key]


def _runner_meta(nc):
    partition_name = nc.partition_id_tensor.name if nc.partition_id_tensor else None
    in_names, out_names, out_avals, zero_outs = [], [], [], []
    import jax
    for alloc in nc.m.functions[0].allocations:
        if not isinstance(alloc, mybir.MemoryLocationSet):
            continue
        name = alloc.memorylocations[0].name
        if alloc.kind == "ExternalInput":
            if name != partition_name:
                in_names.append(name)
        elif alloc.kind == "ExternalOutput":
            out_names.append(name)
            shape = tuple(alloc.tensor_shape)
            dtype = mybir.dt.np(alloc.dtype)
            out_avals.append(jax.core.ShapedArray(shape, dtype))
            zero_outs.append(np.zeros(shape, dtype))
    return partition_name, in_names, out_names, out_avals, zero_outs


_RUNNERS = {}


def _make_runner(nc, chain):
    """Jitted 8-core runner executing the NEFF `chain` times back-to-back
    (iteration i+1 consumes iteration i's outputs as its scratch buffers,
    forcing sequential execution)."""
    key = (id(nc), chain)
    if key in _RUNNERS:
        return _RUNNERS[key]
    import jax
    from jax.sharding import Mesh, PartitionSpec, NamedSharding
    from jax.experimental.shard_map import shard_map
    from concourse.bass2jax import (_bass_exec_p, install_neuronx_cc_hook,
                                    partition_id_tensor)
    install_neuronx_cc_hook()
    partition_name, in_names, out_names, out_avals, zero_outs = _runner_meta(nc)
    n_params = len(in_names)
    all_in = list(in_names) + list(out_names)
    if partition_name is not None:
        all_in = all_in + [partition_name]

    def _body(*args):
        ins = list(args[:n_params])
        cur = list(args[n_params:])
        for _ in range(chain):
            operands = ins + cur
            if partition_name is not None:
                operands = operands + [partition_id_tensor()]
            cur = list(_bass_exec_p.bind(
                *operands,
                out_avals=tuple(out_avals),
                in_names=tuple(all_in),
                out_names=tuple(out_names),
                lowering_input_output_aliases=(),
                sim_require_finite=True,
                sim_require_nnan=True,
                nc=nc,
            ))
        return tuple(cur)

    devices = jax.devices()[:NCORES]
    mesh = Mesh(np.asarray(devices), ("core",))
    nin = n_params + len(out_names)
    sharded = jax.jit(
        shard_map(_body, mesh=mesh,
                  in_specs=(PartitionSpec("core"),) * nin,
                  out_specs=(PartitionSpec("core"),) * len(out_names),
                  check_rep=False),
        keep_unused=True)
    shard = NamedSharding(mesh, PartitionSpec("core"))
    r = (sharded, shard, in_names, out_names, zero_outs)
    _RUNNERS[key] = r
    return r


def _concat_inputs(in_maps, in_names, zero_outs):
    concat_in = [np.concatenate([np.asarray(in_maps[c][n]) for c in range(NCORES)],
                                axis=0) for n in in_names]
    concat_zero = [np.zeros((NCORES * z.shape[0], *z.shape[1:]), z.dtype)
                   for z in zero_outs]
    return concat_in, concat_zero


def kernel(**inputs):
    sc, in_maps = host_prep(inputs)
    nc = _get_program(sc)
    sharded, shard, in_names, out_names, zero_outs = _make_runner(nc, 1)
    concat_in, concat_zero = _concat_inputs(in_maps, in_names, zero_outs)
    out_arrs = sharded(*concat_in, *concat_zero)
    oi = out_names.index("outp")
    full = np.asarray(out_arrs[oi]).reshape(NCORES, BPC, NP, D)[:, :, :N, :]
    return np.ascontiguousarray(full.reshape(B, N, D), dtype=np.float32)


def bench(inputs, chain=65, reps=15):
    """Measure per-execution HW time: bench-variant programs with internal
    (unfed) inputs and an in-program For_i repeat loop; difference R=chain
    vs R=1 wall time to cancel dispatch/transfer overhead."""
    import time
    import jax
    sc, in_maps = host_prep(inputs)

    def _run_R(R):
        key = (tuple(sorted((k, v) for k, v in sc.items())), "bench", R)
        if key not in _CACHE:
            _CACHE[key] = build_program(sc, bench_R=R)
        nc = _CACHE[key]
        sharded, shard, in_names, out_names, zero_outs = _make_runner(nc, 1)
        concat_in, concat_zero = _concat_inputs(
            [dict() for _ in range(NCORES)], in_names, zero_outs)
        out = sharded(*concat_in, *concat_zero)
        jax.block_until_ready(out)
        ts = []
        for _ in range(reps):
            t0 = time.perf_counter()
            out = sharded(*concat_in, *concat_zero)
            jax.block_until_ready(out)
            ts.append(time.perf_counter() - t0)
        return min(ts)

    t1 = _run_R(1)
    tn = _run_R(chain)
    per_exec_ns = (tn - t1) / (chain - 1) * 1e9
    return per_exec_ns, t1, tn



# revision 46
# speedup vs baseline: 1.0866x; 1.0866x over previous
"""Trainium2 Bass kernel: ViT-style transformer block with polynomial attention.

Sharding: pure data-parallel over batch B=32 across 8 NeuronCores (4 batch
elements per core).  No collectives.  Each core computes the full block for
its batch slice; host gathers/concats.

v2 layout strategy (single fused pipeline, per-batch interleaved):
  - tokens padded per-batch 577 -> 640 (5 tiles of 128); 4*640 = 2560/core.
  - attention(b) and the MLP chunks that become ready after batch b are
    emitted in one program-order stream so every engine (esp. PE) stays
    continuously busy: while PE runs MLP(b) matmuls, DVE/ACT run LN/poly
    prep for attention(b+1).
  - LN gains/biases folded into downstream weights on host; on-chip LN is
    bn_stats/bn_aggr (DVE) + Rsqrt (ACT, one op) + mu*rstd (Pool) + one
    fused scale/shift (DVE, 2x mode).
  - transposes: PE-transpose 3x[128,128] chunks into ONE [128,384] PSUM
    tile, then ONE strided 3-way copy to SBUF (halves per-copy overhead).
  - scores per (head, m-chunk) go into a [128,1024] two-bank PSUM tile so
    the PolyAttn Square (+B/2A bias) is ONE ACT op over 578 cols; the fold
    relu(sq + C2A) runs on DVE in 4x bf16 mode (all-SBUF, 2-byte).
  - attn@v accumulates ALL 6 heads into one [128,396] PSUM tile per token
    tile ([64 v | masked-ones | pad] per head), so Z extraction is one
    strided op + one reciprocal; 6 per-head scales produce attn-out.
  - PolyGELU is a pure Square on ACT (scale folded into fc2 weights, the
    constant into fc2 bias via column sums).
  - all matmul operands bf16; fp32 accum in PSUM; residual stream fp32.
"""

import sys

for _p in ("/opt/trn_rl_repo",):
    if _p not in sys.path:
        sys.path.insert(0, _p)

from contextlib import ExitStack

import os

import numpy as np
import ml_dtypes

SIMSAFE = bool(int(os.environ.get("K_SIMSAFE", "0")))

import concourse.bacc as bacc
import concourse.mybir as mybir
import concourse.tile as tile

B, N, D, H = 32, 577, 384, 6
HD = D // H            # 64
HID = 4 * D            # 1536
LN_EPS = 1e-5
ATTN_EPS = 1e-6

NCORES = 8
BPC = B // NCORES      # 4 batches per core
NP = 640               # padded tokens per batch (5 * 128)
NT = NP // 128         # 5 token tiles per batch
TP = BPC * NP          # 2560 tokens per core
GT = TP // 128         # 20 token tiles per core
KC_D = D // 128        # 3 contraction chunks over D
FC_H = HID // 128      # 12 chunks over hidden
NV = N + 1             # 578: even score/poly width covering valid n tokens
VW = HD + 2            # 66: per-head v width ([v | masked-ones | pad])

F32 = mybir.dt.float32
BF16 = mybir.dt.bfloat16
AF = mybir.ActivationFunctionType
ALU = mybir.AluOpType

MT = BF16              # matmul operand dtype
FP8 = mybir.dt.float8e4   # q/k score operands (DoubleRow perf mode)
NPBF = np.dtype(ml_dtypes.bfloat16)


def _ln(nc, st, consts, x_t, out_t, tg):
    """LayerNorm center+scale (gain/bias folded into downstream weights).
    Mean/var in one DVE pass; h = (x - mu) * rstd in one fused op.  Stat
    tile tags are per-callsite (tg) so LN1(b+1) is not ring-serialized
    behind LN2(b)."""
    s6 = st.tile([128, 6], F32, tag="s6" + tg, name="s6")
    nc.vector.bn_stats(s6, x_t)
    mv = st.tile([128, 2], F32, tag="mv" + tg, name="mv")
    nc.vector.bn_aggr(mv, s6)
    sd = st.tile([128, 1], F32, tag="sd" + tg, name="sd")
    nc.scalar.activation(sd, mv[:, 1:2], AF.Sqrt, bias=consts["lneps"])
    rstd = st.tile([128, 1], F32, tag="rstd" + tg, name="rstd")
    nc.vector.reciprocal(rstd, sd)
    nc.vector.tensor_scalar(out=out_t, in0=x_t, scalar1=mv[:, 0:1],
                            scalar2=rstd, op0=ALU.subtract, op1=ALU.mult)


def build_program(sc, bench_R=0):
    """sc: dict of host scalar constants / flags."""
    nc = bacc.Bacc("TRN2", target_bir_lowering=False, debug=False)

    kind_in = "Internal" if bench_R else "ExternalInput"
    xp = nc.dram_tensor("xp", [TP, D], F32, kind=kind_in).ap()
    wqkv_d = nc.dram_tensor("wqkv", [128, KC_D * 3 * D], MT, kind=kind_in).ap()
    wproj_d = nc.dram_tensor("wproj", [128, KC_D * D], MT, kind=kind_in).ap()
    wfc1_d = nc.dram_tensor("wfc1", [128, KC_D * HID], MT, kind=kind_in).ap()
    wfc2_d = nc.dram_tensor("wfc2", [128, FC_H * D], MT, kind=kind_in).ap()
    qkb_d = nc.dram_tensor("qkb", [128, 6], F32, kind=kind_in).ap()
    vbr_d = nc.dram_tensor("vbr", [128, D], F32, kind=kind_in).ap()
    pbr_d = nc.dram_tensor("pbr", [128, D], F32, kind=kind_in).ap()
    f2br_d = nc.dram_tensor("f2br", [128, D], F32, kind=kind_in).ap()
    g1b_d = nc.dram_tensor("g1b", [128, FC_H], F32, kind=kind_in).ap()
    vone6_d = nc.dram_tensor("vone6", [128, NT * H * 2], F32, kind=kind_in).ap()
    ident_d = nc.dram_tensor("ident", [128, 128], MT, kind=kind_in).ap()
    outp = nc.dram_tensor("outp", [128 if bench_R else TP, D], F32,
                          kind="ExternalOutput").ap()

    A2 = sc["B2A"]          # B/(2A): square-pass bias
    C2A = sc["C2A"]         # (C - B^2/(4A))/A: fold add before max(.,0)
    polymode = sc["polymode"]  # "fold" (A>0), "relu" (A<0), "lin" (A==0)
    pC2 = sc["pC2"]
    pA = sc["pA"]
    pB = sc["pB"]
    pC = sc["pC"]
    gelmode = sc["gelmode"]
    add_vb = sc["add_vb"]
    add_pb = sc["add_pb"]
    add_f2b = sc["add_f2b"]
    add_qkb = sc.get("add_qkb", True)

    with ExitStack() as octx:
        tc = octx.enter_context(tile.TileContext(nc))
        cp = octx.enter_context(tc.tile_pool(name="consts", bufs=1))
        # persistent constants — attention-phase tensors first so the PE can
        # start as soon as possible; MLP weights arrive much later.
        # attn-critical consts on the ACT queue (short; LN's Sqrt shares it)
        ident = cp.tile([128, 128], MT, name="ident_sb")
        nc.scalar.dma_start(ident, ident_d)
        wqkv = cp.tile([128, KC_D * 3 * D], MT, name="wqkv_sb")
        for kc in range(KC_D):
            nc.scalar.dma_start(wqkv[:, kc * 3 * D:(kc + 1) * 3 * D],
                                wqkv_d[:, kc * 3 * D:(kc + 1) * 3 * D])
        c_lneps = cp.tile([128, 1], F32, name="c_lneps")
        nc.vector.memset(c_lneps, LN_EPS)
        c_b2a = cp.tile([128, 1], F32, name="c_b2a")
        nc.vector.memset(c_b2a, A2)
        c_pc2 = cp.tile([128, 1], F32, name="c_pc2")
        nc.vector.memset(c_pc2, pC2 if polymode == "relu" else pC)
        consts = {"lneps": c_lneps}
        # the rest goes on the SP queue, issued AFTER batch 0's x prefetch
        # (see _body_phases) so LN1 isn't starved at startup.
        qkb = cp.tile([128, 6], F32, name="qkb_sb")
        vone6 = cp.tile([128, NT * H * 2], F32, name="vone6_sb")
        vbr = cp.tile([128, D], F32, name="vbr_sb")
        wproj = cp.tile([128, KC_D * D], MT, name="wproj_sb")
        pbr = cp.tile([128, D], F32, name="pbr_sb")
        wfc1 = cp.tile([128, KC_D * HID], MT, name="wfc1_sb")
        g1b = cp.tile([128, FC_H], F32, name="g1b_sb")
        wfc2 = cp.tile([128, FC_H * D], MT, name="wfc2_sb")
        f2br = cp.tile([128, D], F32, name="f2br_sb")

        def _late_const_dmas():
            # Pool SWDGE queue: keeps the SP queue free for x prefetches
            nc.gpsimd.dma_start(qkb, qkb_d)
            nc.gpsimd.dma_start(vone6, vone6_d)
            nc.gpsimd.dma_start(vbr, vbr_d)
            nc.gpsimd.dma_start(wproj, wproj_d)
            nc.gpsimd.dma_start(pbr, pbr_d)
            nc.gpsimd.dma_start(wfc1, wfc1_d)
            nc.gpsimd.dma_start(g1b, g1b_d)
            nc.gpsimd.dma_start(wfc2, wfc2_d)
            nc.gpsimd.dma_start(f2br, f2br_d)

        def _body_phases():
            with ExitStack() as actx:
                # PSUM: mm(2) + tp(2) + sc(2x2) + av(2) = 8 banks
                pp = actx.enter_context(tc.tile_pool(name="pp", bufs=1,
                                                     space="PSUM"))
                st = actx.enter_context(tc.tile_pool(name="st", bufs=4))
                xpool = actx.enter_context(tc.tile_pool(name="xp", bufs=2))
                hpool = actx.enter_context(tc.tile_pool(name="hp", bufs=NT + 1))
                hTp = actx.enter_context(tc.tile_pool(name="hT", bufs=2))
                qkp = actx.enter_context(tc.tile_pool(name="qk", bufs=12))
                vp = actx.enter_context(tc.tile_pool(name="vp", bufs=2 * NT))
                tsbp = actx.enter_context(tc.tile_pool(name="tsb", bufs=2))
                polyp = actx.enter_context(tc.tile_pool(name="poly",
                                                        bufs=H * NT))
                aop = actx.enter_context(tc.tile_pool(name="ao", bufs=NT + 1))
                aTp = actx.enter_context(tc.tile_pool(name="aT", bufs=2))
                x2p = actx.enter_context(tc.tile_pool(name="x2p", bufs=GT))
                h2Tp = actx.enter_context(tc.tile_pool(name="h2T", bufs=1))
                gp = actx.enter_context(tc.tile_pool(name="gp",
                                                     bufs=FC_H + 1))
                outpl = actx.enter_context(tc.tile_pool(name="outl", bufs=1))

                x2_tiles = [None] * GT
                # h2T persists across batches: MLP chunks cross batch bounds
                h2T = h2Tp.tile([128, KC_D * TP], MT, name="h2T_all")
                h2T3 = h2T.rearrange("p (k c) -> p k c", c=TP)
                mlp_done = [0]  # chunks of 512 tokens emitted so far
                x_tiles = {}
                # v ring: 10 explicit tiles (2 sets of NT); the masked-ones
                # columns depend only on tt, so write them once up front
                # (AFTER the vone6 DMA is emitted — reads bind to prior writes)
                v_ring = []

                def _init_v_ring():
                    for s in range(2 * NT):
                        vt = vp.tile([128, H * VW], MT, tag=f"v{s}",
                                     name=f"v_{s}", bufs=1)
                        v_ring.append(vt)
                        nc.gpsimd.tensor_copy(
                            vt.rearrange("p (h c) -> p h c", c=VW)
                            [:, :, HD:HD + 2],
                            vone6[:, (s % NT) * H * 2:(s % NT + 1) * H * 2]
                            .rearrange("p (h c) -> p h c", c=2))

                def prefetch_x(b):
                    xa = xpool.tile([128, NT * D], F32, tag="x", name="x_all")
                    xa3 = xa.rearrange("p (t c) -> p t c", c=D)
                    src_v = xp[b * NP:(b + 1) * NP, :].rearrange(
                        "(t p) c -> p t c", p=128)
                    nc.sync.dma_start(xa3, src_v)
                    x_tiles[b] = [xa3[:, tt, :] for tt in range(NT)]

                def transpose3(src_t, dst3, dst_col, engine):
                    """Transpose [128(tok), 384(feat)] -> dst3[:, k, col:+128].
                    "act"/"vec": PE-transpose 3 chunks into one [128,384] psum
                    + ONE strided 3-way copy.  "dma": DMA crossbar (latency-
                    tolerant consumers only), issued on the ACT queue."""
                    if engine == "dma":
                        nc.sync.dma_start_transpose(
                            dst3[:, :, dst_col:dst_col + 128], src_t)
                        return
                    tp_ps = pp.tile([128, KC_D * 128], src_t.dtype, tag="fr",
                                    name="tp_ps", space="PSUM", bufs=2)
                    for kc in range(KC_D):
                        nc.tensor.transpose(
                            tp_ps[:, kc * 128:(kc + 1) * 128],
                            src_t[:, kc * 128:(kc + 1) * 128], ident)
                    tp3 = tp_ps.rearrange("p (k c) -> p k c", c=128)
                    dview = dst3[:, :, dst_col:dst_col + 128]
                    if engine == "act":
                        nc.scalar.activation(dview, tp3, AF.Copy)
                    else:
                        nc.vector.tensor_copy(dview, tp3)

                def attention(b, fill):
                    if b + 1 < BPC:
                        prefetch_x(b + 1)
                    # --- LN1 + transpose to feature-major hT ---
                    hT = hTp.tile([128, KC_D * NP], MT, tag="hT",
                                  name=f"hT{b}")
                    hT3 = hT.rearrange("p (k c) -> p k c", c=NP)
                    x_ts = x_tiles.pop(b)
                    h_ts = []
                    with tc.high_priority(1200):
                        for tt in range(NT):
                            h_t = hpool.tile([128, D], MT, tag="h", name="h_t")
                            h_ts.append(h_t)
                            _ln(nc, st, consts, x_ts[tt], h_t, "1")
                    fill(3)
                    with tc.high_priority(1200):
                        for tt in range(NT):
                            transpose3(h_ts[tt], hT3, tt * 128, "act")

                    # --- QKV (q,k feature-major; fc order lets hp0 start early)
                    qk = [None] * 6
                    for fc in (0, 3, 1, 4, 2, 5):
                        t = qkp.tile([128, NP], MT, tag="qk", name=f"qk{b}_{fc}")
                        qk[fc] = t
                        for c0, c1 in ((0, 512), (512, NP)):
                            ps = pp.tile([128, 512], F32, tag="fr", name="qk_ps",
                                         space="PSUM", bufs=2)
                            for kc in range(KC_D):
                                nc.tensor.matmul(
                                    ps[:, 0:c1 - c0],
                                    wqkv[:, kc * 3 * D + fc * 128:
                                         kc * 3 * D + fc * 128 + 128],
                                    hT3[:, kc, c0:c1],
                                    start=(kc == 0), stop=(kc == KC_D - 1),
                                )
                            if add_qkb:
                                nc.vector.tensor_scalar(
                                    out=t[:, c0:c1], in0=ps[:, 0:c1 - c0],
                                    scalar1=qkb[:, fc:fc + 1], scalar2=None,
                                    op0=ALU.add)
                            else:
                                nc.vector.tensor_copy(t[:, c0:c1],
                                                      ps[:, 0:c1 - c0])

                    fill(1)
                    # --- v token-major [128, H*VW]: [v | masked-ones | pad]
                    v_sb = []
                    for tt in range(NT):
                        ps = pp.tile([128, 512], F32, tag="fr", name="v_ps",
                                     space="PSUM", bufs=2)
                        for kc in range(KC_D):
                            nc.tensor.matmul(
                                ps[:, 0:D],
                                hT3[:, kc, tt * 128:(tt + 1) * 128],
                                wqkv[:, kc * 3 * D + 768:kc * 3 * D + 1152],
                                start=(kc == 0), stop=(kc == KC_D - 1),
                            )
                        vt = v_ring[(b % 2) * NT + tt]
                        v_sb.append(vt)
                        v3 = vt.rearrange("p (h c) -> p h c", c=VW)
                        ps3 = ps[:, 0:D].rearrange("p (h c) -> p h c", c=HD)
                        if add_vb:
                            nc.vector.tensor_add(
                                v3[:, :, 0:HD], ps3,
                                vbr.rearrange("p (h c) -> p h c", c=HD))
                            if tt == NT - 1:
                                nc.vector.tensor_scalar(
                                    out=v3[:, :, 0:HD], in0=v3[:, :, 0:HD],
                                    scalar1=vone6[:, tt * H * 2:tt * H * 2 + 1],
                                    scalar2=None, op0=ALU.mult)
                        else:
                            # padded-token rows of psum are exactly 0 (h_pad==0)
                            nc.vector.tensor_copy(v3[:, :, 0:HD], ps3)

                    fill(1)
                    # --- scores + poly: all 6 heads (pair-interleaved PE rows)
                    # high priority: the ACT square stretch paces the whole
                    # batch; score matmuls must preempt interleaved MLP work
                    polys = {}
                    for hp in range(H // 2):
                        h0, h1 = 2 * hp, 2 * hp + 1
                        for h in (h0, h1):
                            for mc in range(NT):
                                polys[(h, mc)] = polyp.tile(
                                    [128, NP], MT, tag="poly",
                                    name=f"poly{h}_{mc}")
                                if SIMSAFE:
                                    # pad cols are never consumed (masked via
                                    # v rows); init only for CoreSim checks
                                    nc.vector.memset(
                                        polys[(h, mc)][:, NV:NP], 0.0)
                        prio_ctx = tc.high_priority(1500)
                        prio_ctx.__enter__()
                        for mc in range(NT):
                            pss = {}
                            for h in (h0, h1):
                                fcq = h // 2
                                row = (h % 2) * 64
                                ps = pp.tile([128, 1024], F32, tag="sc",
                                             name="sc_ps", space="PSUM",
                                             bufs=2)
                                pss[h] = ps
                                for c0, c1 in ((0, 512), (512, NV)):
                                    nc.tensor.matmul(
                                        ps[:, c0:c1],
                                        qk[3 + fcq][row:row + 64,
                                                    mc * 128:(mc + 1) * 128],
                                        qk[fcq][row:row + 64, c0:c1],
                                        start=True, stop=True,
                                    )
                            for h in (h0, h1):
                                if polymode == "lin":
                                    nc.scalar.activation(
                                        polys[(h, mc)][:, 0:NV],
                                        pss[h][:, 0:NV], AF.Relu,
                                        scale=pB, bias=c_pc2)
                                elif polymode == "relu":
                                    tsb = tsbp.tile([128, NP], BF16, tag="tsb",
                                                    name="t_sb")
                                    nc.scalar.activation(
                                        tsb[:, 0:NV], pss[h][:, 0:NV],
                                        AF.Square, bias=c_b2a)
                                    nc.scalar.activation(
                                        polys[(h, mc)][:, 0:NV],
                                        tsb[:, 0:NV], AF.Relu,
                                        scale=pA, bias=c_pc2)
                                else:  # fold
                                    tsb = tsbp.tile([128, NP], BF16, tag="tsb",
                                                    name="t_sb")
                                    nc.scalar.activation(
                                        tsb[:, 0:NV], pss[h][:, 0:NV],
                                        AF.Square, bias=c_b2a)
                                    # DVE 4x mode: all-SBUF, 2-byte, packed
                                    nc.vector.tensor_scalar(
                                        out=polys[(h, mc)][:, 0:NV],
                                        in0=tsb[:, 0:NV], scalar1=C2A,
                                        scalar2=0.0, op0=ALU.add, op1=ALU.max)
                        prio_ctx.__exit__(None, None, None)
                        fill(2)

                    # --- attn @ [v | ones]: all heads in one [128,396] psum
                    ao_t = []
                    for nt in range(NT):
                        ps = pp.tile([128, H * VW], F32, tag="mm",
                                     name="av_ps", space="PSUM", bufs=2)
                        for h in range(H):
                            off = h * VW
                            for mc in range(NT):
                                nc.tensor.matmul(
                                    ps[:, off:off + VW],
                                    polys[(h, mc)][:, nt * 128:(nt + 1) * 128],
                                    v_sb[mc][:, off:off + VW],
                                    start=(mc == 0), stop=(mc == NT - 1),
                                )
                        # one copy frees the psum bank; normalize from SBUF
                        avs = st.tile([128, H * VW], F32, tag="avs", name="avs",
                                      bufs=2)
                        nc.vector.tensor_copy(avs, ps)
                        zv = avs.rearrange("p (h c) -> p h c", c=VW)[:, :, HD]
                        zt = st.tile([128, H], F32, tag="zt", name="zt")
                        nc.vector.tensor_scalar_add(zt, zv, float(sc["eps_eff"]))
                        rz = st.tile([128, H], F32, tag="rz", name="rz")
                        nc.vector.reciprocal(rz, zt)
                        ao = aop.tile([128, D], MT, tag="ao", name=f"ao{b}_{nt}")
                        ao_t.append(ao)
                        for h in range(H):
                            eng = nc.vector if h < 3 else nc.gpsimd
                            eng.tensor_scalar(
                                out=ao[:, h * HD:(h + 1) * HD],
                                in0=avs[:, h * VW:h * VW + HD],
                                scalar1=rz[:, h:h + 1], scalar2=None,
                                op0=ALU.mult)

                    # --- transpose attn_out, proj, residual, LN2 (fused) ---
                    aT = aTp.tile([128, KC_D * NP], MT, tag="aT", name=f"aT{b}")
                    aT3 = aT.rearrange("p (k c) -> p k c", c=NP)
                    for nt in range(NT):
                        transpose3(ao_t[nt], aT3, nt * 128, "act")
                    for tt in range(NT):
                        gt = b * NT + tt
                        ps = pp.tile([128, 512], F32, tag="mm", name="pj_ps",
                                     space="PSUM", bufs=2)
                        for kc in range(KC_D):
                            nc.tensor.matmul(
                                ps[:, 0:D],
                                aT3[:, kc, tt * 128:(tt + 1) * 128],
                                wproj[:, kc * D:(kc + 1) * D],
                                start=(kc == 0), stop=(kc == KC_D - 1),
                            )
                        x2t = x2p.tile([128, D], F32, tag="x2", name=f"x2_{gt}")
                        x2_tiles[gt] = x2t
                        if add_pb:
                            nc.vector.tensor_add(x2t, ps[:, 0:D], pbr)
                            nc.vector.tensor_add(x2t, x2t, x_ts[tt])
                        else:
                            nc.vector.tensor_add(x2t, ps[:, 0:D], x_ts[tt])
                        # LN2 per tile right after its residual: the h2T
                        # transpose is in flight while the next proj runs
                        h2_t = hpool.tile([128, D], MT, tag="h2", name="h2_t")
                        _ln(nc, st, consts, x2t, h2_t, "2")
                        transpose3(h2_t, h2T3, gt * 128, "act")

                def mlp_chunk(nch):
                    """Generator: yields after each piece so the caller can
                    interleave MLP work into the next batch's attention."""
                    g_sb = []
                    for fc in range(FC_H):
                        ps = pp.tile([128, 512], F32, tag="mm", name="f1_ps",
                                     space="PSUM", bufs=2)
                        for kc in range(KC_D):
                            nc.tensor.matmul(
                                ps,
                                wfc1[:, kc * HID + fc * 128:
                                     kc * HID + fc * 128 + 128],
                                h2T3[:, kc, nch * 512:(nch + 1) * 512],
                                start=(kc == 0), stop=(kc == KC_D - 1),
                            )
                        gt_sb = gp.tile([128, 512], MT, tag="g",
                                        name=f"g{nch}_{fc}")
                        g_sb.append(gt_sb)
                        nc.scalar.activation(
                            gt_sb, ps,
                            AF.Square if gelmode == "quad" else AF.Identity,
                            bias=g1b[:, fc:fc + 1])
                        if fc % 3 == 2:
                            yield
                    ota = outpl.tile([128, 4 * D], F32, tag="ot", name="out_a")
                    ota3 = ota.rearrange("p (t c) -> p t c", c=D)
                    for tt in range(4):
                        gt = nch * 4 + tt
                        ps = pp.tile([128, 512], F32, tag="mm", name="f2_ps",
                                     space="PSUM", bufs=2)
                        for fc in range(FC_H):
                            nc.tensor.matmul(
                                ps[:, 0:D],
                                g_sb[fc][:, tt * 128:(tt + 1) * 128],
                                wfc2[:, fc * D:(fc + 1) * D],
                                start=(fc == 0), stop=(fc == FC_H - 1),
                            )
                        ot = ota3[:, tt, :]
                        if add_f2b:
                            nc.vector.tensor_add(ot, ps[:, 0:D], f2br)
                            nc.vector.tensor_add(ot, ot, x2_tiles[gt])
                        else:
                            nc.vector.tensor_add(ot, ps[:, 0:D], x2_tiles[gt])
                        yield
                    # out DMA on the Pool SWDGE queue: SP head-of-line
                    # blocking would park x-prefetch/h2T behind this wait
                    if bench_R:
                        nc.gpsimd.dma_start(outp[0:128, :], ota3[:, 0, :])
                    else:
                        dst_v = outp[nch * 512:(nch + 1) * 512, :].rearrange(
                            "(t p) c -> p t c", p=128)
                        nc.gpsimd.dma_start(dst_v, ota3)

                prefetch_x(0)
                _late_const_dmas()
                _init_v_ring()
                pending = []  # MLP piece generators ready for interleaving

                def fill(n):
                    for _ in range(n):
                        while pending:
                            if next(pending[0], StopIteration) is StopIteration:
                                pending.pop(0)
                                continue
                            break
                        else:
                            return

                for b in range(BPC):
                    attention(b, fill)
                    fill(10 ** 6)  # drain leftovers
                    ready = ((b + 1) * NP) // 512
                    while mlp_done[0] < ready:
                        pending.append(mlp_chunk(mlp_done[0]))
                        mlp_done[0] += 1
                fill(10 ** 6)

        if bench_R:
            with tc.For_i(0, bench_R, 1):
                _body_phases()
        else:
            _body_phases()

    nc.compile()
    return nc


def host_prep(inputs):
    """Fold LN affine params into weights, build per-core input maps."""
    f = lambda k: np.asarray(inputs[k], dtype=np.float32)
    x = f("x")
    qkv_w, qkv_b = f("qkv_w"), f("qkv_b")
    proj_w, proj_b = f("proj_w"), f("proj_b")
    fc1_w, fc1_b = f("fc1_w"), f("fc1_b")
    fc2_w, fc2_b = f("fc2_w"), f("fc2_b")
    ln1_g, ln1_b = f("ln1_g"), f("ln1_b")
    ln2_g, ln2_b = f("ln2_g"), f("ln2_b")
    attn_abc = f("attn_abc")
    gelu_abc = f("gelu_abc")

    scale = HD ** -0.5
    pA = float(attn_abc[0]) * scale * scale
    pB = float(attn_abc[1]) * scale
    pC = float(attn_abc[2])
    sc = {"pA": pA, "pB": pB, "pC": pC}
    if pA > 1e-12:
        sc["polymode"] = "fold"
        sc["B2A"] = pB / (2.0 * pA)
        sc["pC2"] = pC - pB * pB / (4.0 * pA)
        sc["C2A"] = sc["pC2"] / pA
        eps_eff = ATTN_EPS / pA
    elif pA < -1e-12:
        sc["polymode"] = "relu"
        sc["B2A"] = pB / (2.0 * pA)
        sc["pC2"] = pC - pB * pB / (4.0 * pA)
        sc["C2A"] = 0.0
        eps_eff = ATTN_EPS
    else:
        sc["polymode"] = "lin"
        sc["B2A"] = 0.0
        sc["pC2"] = pC
        sc["C2A"] = 0.0
        eps_eff = ATTN_EPS

    ga, gb, gc = float(gelu_abc[0]), float(gelu_abc[1]), float(gelu_abc[2])
    fc1_b_eff = (fc1_b + ln2_b @ fc1_w).astype(np.float32)
    if abs(ga) > 1e-12:
        # gelu(u) = ga*(u + gb/2ga)^2 + (gc - gb^2/4ga); fold ga into fc2_w
        # and the constant into fc2_b via column sums.
        sc["gelmode"] = "quad"
        gC2 = gc - gb * gb / (4.0 * ga)
        g1bias = fc1_b_eff + gb / (2.0 * ga)
        fc2_w_eff = ga * fc2_w
        fc2_b_eff = fc2_b + gC2 * fc2_w.sum(axis=0)
    else:
        # gelu(u) = gb*u + gc
        sc["gelmode"] = "lin"
        g1bias = fc1_b_eff
        fc2_w_eff = gb * fc2_w
        fc2_b_eff = fc2_b + gc * fc2_w.sum(axis=0)

    sc["eps_eff"] = float(eps_eff)
    qkv_w_eff = (ln1_g[:, None] * qkv_w).astype(np.float32)
    qkv_b_eff = (qkv_b + ln1_b @ qkv_w).astype(np.float32)
    fc1_w_eff = (ln2_g[:, None] * fc1_w).astype(np.float32)

    sc["add_vb"] = bool(np.any(qkv_b_eff[2 * D:] != 0.0))
    sc["add_pb"] = bool(np.any(proj_b != 0.0))
    sc["add_f2b"] = bool(np.any(fc2_b_eff != 0.0))
    sc["add_qkb"] = bool(np.any(qkv_b_eff[:2 * D] != 0.0))

    common = {
        "wqkv": np.ascontiguousarray(
            qkv_w_eff.reshape(KC_D, 128, 3 * D).transpose(1, 0, 2)
            .reshape(128, KC_D * 3 * D)).astype(NPBF),
        "wproj": np.ascontiguousarray(
            proj_w.reshape(KC_D, 128, D).transpose(1, 0, 2)
            .reshape(128, KC_D * D)).astype(NPBF),
        "wfc1": np.ascontiguousarray(
            fc1_w_eff.reshape(KC_D, 128, HID).transpose(1, 0, 2)
            .reshape(128, KC_D * HID)).astype(NPBF),
        "wfc2": np.ascontiguousarray(
            fc2_w_eff.astype(np.float32).reshape(FC_H, 128, D).transpose(1, 0, 2)
            .reshape(128, FC_H * D)).astype(NPBF),
        "qkb": np.ascontiguousarray(qkv_b_eff[:2 * D].reshape(6, 128).T),
        "vbr": np.ascontiguousarray(
            np.broadcast_to(qkv_b_eff[2 * D:], (128, D))),
        "pbr": np.ascontiguousarray(np.broadcast_to(proj_b, (128, D))),
        "f2br": np.ascontiguousarray(np.broadcast_to(fc2_b_eff, (128, D))),
        "g1b": np.ascontiguousarray(g1bias.reshape(FC_H, 128).T),
        "vone6": None,
        "ident": np.eye(128, dtype=np.float32).astype(NPBF),
    }
    mask = (np.arange(NP) < N).astype(np.float32)  # [640]
    mz = np.zeros((NT, 128, H, 2), np.float32)
    mz[:, :, :, 0] = mask.reshape(NT, 128)[:, :, None]
    common["vone6"] = np.ascontiguousarray(
        mz.transpose(1, 0, 2, 3).reshape(128, NT * H * 2))
    common = {k: (np.ascontiguousarray(v, dtype=np.float32)
                  if v.dtype != NPBF else v)
              for k, v in common.items()}

    in_maps = []
    for c in range(NCORES):
        xp_c = np.zeros((BPC, NP, D), np.float32)
        xp_c[:, :N, :] = x[c * BPC:(c + 1) * BPC]
        m = dict(common)
        m["xp"] = xp_c.reshape(TP, D)
        in_maps.append(m)
    return sc, in_maps


_CACHE = {}


def _get_program(sc):
    key = tuple(sorted((k, v) for k, v in sc.items()))
    if key not in _CACHE:
        _CACHE[key] = build_program(sc)
    return _CACHE[key]


def _runner_meta(nc):
    partition_name = nc.partition_id_tensor.name if nc.partition_id_tensor else None
    in_names, out_names, out_avals, zero_outs = [], [], [], []
    import jax
    for alloc in nc.m.functions[0].allocations:
        if not isinstance(alloc, mybir.MemoryLocationSet):
            continue
        name = alloc.memorylocations[0].name
        if alloc.kind == "ExternalInput":
            if name != partition_name:
                in_names.append(name)
        elif alloc.kind == "ExternalOutput":
            out_names.append(name)
            shape = tuple(alloc.tensor_shape)
            dtype = mybir.dt.np(alloc.dtype)
            out_avals.append(jax.core.ShapedArray(shape, dtype))
            zero_outs.append(np.zeros(shape, dtype))
    return partition_name, in_names, out_names, out_avals, zero_outs


_RUNNERS = {}


def _make_runner(nc, chain):
    """Jitted 8-core runner executing the NEFF `chain` times back-to-back
    (iteration i+1 consumes iteration i's outputs as its scratch buffers,
    forcing sequential execution)."""
    key = (id(nc), chain)
    if key in _RUNNERS:
        return _RUNNERS[key]
    import jax
    from jax.sharding import Mesh, PartitionSpec, NamedSharding
    from jax.experimental.shard_map import shard_map
    from concourse.bass2jax import (_bass_exec_p, install_neuronx_cc_hook,
                                    partition_id_tensor)
    install_neuronx_cc_hook()
    partition_name, in_names, out_names, out_avals, zero_outs = _runner_meta(nc)
    n_params = len(in_names)
    all_in = list(in_names) + list(out_names)
    if partition_name is not None:
        all_in = all_in + [partition_name]

    def _body(*args):
        ins = list(args[:n_params])
        cur = list(args[n_params:])
        for _ in range(chain):
            operands = ins + cur
            if partition_name is not None:
                operands = operands + [partition_id_tensor()]
            cur = list(_bass_exec_p.bind(
                *operands,
                out_avals=tuple(out_avals),
                in_names=tuple(all_in),
                out_names=tuple(out_names),
                lowering_input_output_aliases=(),
                sim_require_finite=True,
                sim_require_nnan=True,
                nc=nc,
            ))
        return tuple(cur)

    devices = jax.devices()[:NCORES]
    mesh = Mesh(np.asarray(devices), ("core",))
    nin = n_params + len(out_names)
    sharded = jax.jit(
        shard_map(_body, mesh=mesh,
                  in_specs=(PartitionSpec("core"),) * nin,
                  out_specs=(PartitionSpec("core"),) * len(out_names),
                  check_rep=False),
        keep_unused=True)
    shard = NamedSharding(mesh, PartitionSpec("core"))
    r = (sharded, shard, in_names, out_names, zero_outs)
    _RUNNERS[key] = r
    return r


def _concat_inputs(in_maps, in_names, zero_outs):
    concat_in = [np.concatenate([np.asarray(in_maps[c][n]) for c in range(NCORES)],
                                axis=0) for n in in_names]
    concat_zero = [np.zeros((NCORES * z.shape[0], *z.shape[1:]), z.dtype)
                   for z in zero_outs]
    return concat_in, concat_zero


def kernel(**inputs):
    sc, in_maps = host_prep(inputs)
    nc = _get_program(sc)
    sharded, shard, in_names, out_names, zero_outs = _make_runner(nc, 1)
    concat_in, concat_zero = _concat_inputs(in_maps, in_names, zero_outs)
    out_arrs = sharded(*concat_in, *concat_zero)
    oi = out_names.index("outp")
    full = np.asarray(out_arrs[oi]).reshape(NCORES, BPC, NP, D)[:, :, :N, :]
    return np.ascontiguousarray(full.reshape(B, N, D), dtype=np.float32)


def bench(inputs, chain=129, reps=12):
    """Measure per-execution HW time: bench-variant programs with internal
    (unfed) inputs and an in-program For_i repeat loop; difference R=chain
    vs R=1 wall time to cancel dispatch/transfer overhead."""
    import time
    import jax
    sc, in_maps = host_prep(inputs)

    def _run_R(R):
        key = (tuple(sorted((k, v) for k, v in sc.items())), "bench", R)
        if key not in _CACHE:
            _CACHE[key] = build_program(sc, bench_R=R)
        nc = _CACHE[key]
        sharded, shard, in_names, out_names, zero_outs = _make_runner(nc, 1)
        concat_in, concat_zero = _concat_inputs(
            [dict() for _ in range(NCORES)], in_names, zero_outs)
        out = sharded(*concat_in, *concat_zero)
        jax.block_until_ready(out)
        ts = []
        for _ in range(reps):
            t0 = time.perf_counter()
            out = sharded(*concat_in, *concat_zero)
            jax.block_until_ready(out)
            ts.append(time.perf_counter() - t0)
        return min(ts)

    t1 = _run_R(1)
    tn = _run_R(chain)
    per_exec_ns = (tn - t1) / (chain - 1) * 1e9
    return per_exec_ns, t1, tn


# revision 47
# speedup vs baseline: 1.2902x; 1.1874x over previous
"""Trainium2 Bass kernel: ViT-style transformer block with polynomial attention.

Sharding: pure data-parallel over batch B=32 across 8 NeuronCores (4 batch
elements per core).  No collectives.  Each core computes the full block for
its batch slice; host gathers/concats.

v2 layout strategy (single fused pipeline, per-batch interleaved):
  - tokens padded per-batch 577 -> 640 (5 tiles of 128); 4*640 = 2560/core.
  - attention(b) and the MLP chunks that become ready after batch b are
    emitted in one program-order stream so every engine (esp. PE) stays
    continuously busy: while PE runs MLP(b) matmuls, DVE/ACT run LN/poly
    prep for attention(b+1).
  - LN gains/biases folded into downstream weights on host; on-chip LN is
    bn_stats/bn_aggr (DVE) + Rsqrt (ACT, one op) + mu*rstd (Pool) + one
    fused scale/shift (DVE, 2x mode).
  - transposes: PE-transpose 3x[128,128] chunks into ONE [128,384] PSUM
    tile, then ONE strided 3-way copy to SBUF (halves per-copy overhead).
  - scores per (head, m-chunk) go into a [128,1024] two-bank PSUM tile so
    the PolyAttn Square (+B/2A bias) is ONE ACT op over 578 cols; the fold
    relu(sq + C2A) runs on DVE in 4x bf16 mode (all-SBUF, 2-byte).
  - attn@v accumulates ALL 6 heads into one [128,396] PSUM tile per token
    tile ([64 v | masked-ones | pad] per head), so Z extraction is one
    strided op + one reciprocal; 6 per-head scales produce attn-out.
  - PolyGELU is a pure Square on ACT (scale folded into fc2 weights, the
    constant into fc2 bias via column sums).
  - all matmul operands bf16; fp32 accum in PSUM; residual stream fp32.
"""

import sys

for _p in ("/opt/trn_rl_repo",):
    if _p not in sys.path:
        sys.path.insert(0, _p)

from contextlib import ExitStack

import os

import numpy as np
import ml_dtypes

SIMSAFE = bool(int(os.environ.get("K_SIMSAFE", "0")))

import concourse.bacc as bacc
import concourse.mybir as mybir
import concourse.tile as tile

B, N, D, H = 32, 577, 384, 6
HD = D // H            # 64
HID = 4 * D            # 1536
LN_EPS = 1e-5
ATTN_EPS = 1e-6

NCORES = 8
BPC = B // NCORES      # 4 batches per core
NP = 640               # padded tokens per batch (5 * 128)
NT = NP // 128         # 5 token tiles per batch
TP = BPC * NP          # 2560 tokens per core
GT = TP // 128         # 20 token tiles per core
KC_D = D // 128        # 3 contraction chunks over D
FC_H = HID // 128      # 12 chunks over hidden
NV = N + 1             # 578: even score/poly width covering valid n tokens
VW = HD + 2            # 66: per-head v width ([v | masked-ones | pad])

F32 = mybir.dt.float32
BF16 = mybir.dt.bfloat16
AF = mybir.ActivationFunctionType
ALU = mybir.AluOpType

MT = BF16              # matmul operand dtype
FP8 = mybir.dt.float8e4   # q/k score operands (DoubleRow perf mode)
NPBF = np.dtype(ml_dtypes.bfloat16)


def _ln(nc, st, consts, x_t, out_t, tg):
    """LayerNorm center+scale (gain/bias folded into downstream weights).
    Mean/var in one DVE pass; h = (x - mu) * rstd in one fused op.  Stat
    tile tags are per-callsite (tg) so LN1(b+1) is not ring-serialized
    behind LN2(b)."""
    s6 = st.tile([128, 6], F32, tag="s6" + tg, name="s6")
    nc.vector.bn_stats(s6, x_t)
    mv = st.tile([128, 2], F32, tag="mv" + tg, name="mv")
    nc.vector.bn_aggr(mv, s6)
    sd = st.tile([128, 1], F32, tag="sd" + tg, name="sd")
    nc.scalar.activation(sd, mv[:, 1:2], AF.Sqrt, bias=consts["lneps"])
    rstd = st.tile([128, 1], F32, tag="rstd" + tg, name="rstd")
    nc.vector.reciprocal(rstd, sd)
    nc.vector.tensor_scalar(out=out_t, in0=x_t, scalar1=mv[:, 0:1],
                            scalar2=rstd, op0=ALU.subtract, op1=ALU.mult)


def build_program(sc, bench_R=0):
    """sc: dict of host scalar constants / flags."""
    nc = bacc.Bacc("TRN2", target_bir_lowering=False, debug=False)

    kind_in = "Internal" if bench_R else "ExternalInput"
    xp = nc.dram_tensor("xp", [TP, D], F32, kind=kind_in).ap()
    wqkv_d = nc.dram_tensor("wqkv", [128, KC_D * 3 * D], MT, kind=kind_in).ap()
    wproj_d = nc.dram_tensor("wproj", [128, KC_D * D], MT, kind=kind_in).ap()
    wfc1_d = nc.dram_tensor("wfc1", [128, KC_D * HID], MT, kind=kind_in).ap()
    wfc2_d = nc.dram_tensor("wfc2", [128, FC_H * D], MT, kind=kind_in).ap()
    qkb_d = nc.dram_tensor("qkb", [128, 6], F32, kind=kind_in).ap()
    vbr_d = nc.dram_tensor("vbr", [128, D], F32, kind=kind_in).ap()
    pbr_d = nc.dram_tensor("pbr", [128, D], F32, kind=kind_in).ap()
    f2br_d = nc.dram_tensor("f2br", [128, D], F32, kind=kind_in).ap()
    g1b_d = nc.dram_tensor("g1b", [128, FC_H], F32, kind=kind_in).ap()
    vone6_d = nc.dram_tensor("vone6", [128, NT * H * 2], F32, kind=kind_in).ap()
    ident_d = nc.dram_tensor("ident", [128, 128], MT, kind=kind_in).ap()
    outp = nc.dram_tensor("outp", [128 if bench_R else TP, D], F32,
                          kind="ExternalOutput").ap()

    A2 = sc["B2A"]          # B/(2A): square-pass bias
    C2A = sc["C2A"]         # (C - B^2/(4A))/A: fold add before max(.,0)
    polymode = sc["polymode"]  # "fold" (A>0), "relu" (A<0), "lin" (A==0)
    pC2 = sc["pC2"]
    pA = sc["pA"]
    pB = sc["pB"]
    pC = sc["pC"]
    gelmode = sc["gelmode"]
    add_vb = sc["add_vb"]
    add_pb = sc["add_pb"]
    add_f2b = sc["add_f2b"]
    add_qkb = sc.get("add_qkb", True)

    with ExitStack() as octx:
        tc = octx.enter_context(tile.TileContext(nc))
        cp = octx.enter_context(tc.tile_pool(name="consts", bufs=1))
        # persistent constants — attention-phase tensors first so the PE can
        # start as soon as possible; MLP weights arrive much later.
        # attn-critical consts on the ACT queue (short; LN's Sqrt shares it)
        ident = cp.tile([128, 128], MT, name="ident_sb")
        nc.scalar.dma_start(ident, ident_d)
        wqkv = cp.tile([128, KC_D * 3 * D], MT, name="wqkv_sb")
        for kc in range(KC_D):
            nc.scalar.dma_start(wqkv[:, kc * 3 * D:(kc + 1) * 3 * D],
                                wqkv_d[:, kc * 3 * D:(kc + 1) * 3 * D])
        c_lneps = cp.tile([128, 1], F32, name="c_lneps")
        nc.vector.memset(c_lneps, LN_EPS)
        c_b2a = cp.tile([128, 1], F32, name="c_b2a")
        nc.vector.memset(c_b2a, A2)
        c_pc2 = cp.tile([128, 1], F32, name="c_pc2")
        nc.vector.memset(c_pc2, pC2 if polymode == "relu" else pC)
        consts = {"lneps": c_lneps}
        # the rest goes on the SP queue, issued AFTER batch 0's x prefetch
        # (see _body_phases) so LN1 isn't starved at startup.
        qkb = cp.tile([128, 6], F32, name="qkb_sb")
        vone6 = cp.tile([128, NT * H * 2], F32, name="vone6_sb")
        vbr = cp.tile([128, D], F32, name="vbr_sb")
        wproj = cp.tile([128, KC_D * D], MT, name="wproj_sb")
        pbr = cp.tile([128, D], F32, name="pbr_sb")
        wfc1 = cp.tile([128, KC_D * HID], MT, name="wfc1_sb")
        g1b = cp.tile([128, FC_H], F32, name="g1b_sb")
        wfc2 = cp.tile([128, FC_H * D], MT, name="wfc2_sb")
        f2br = cp.tile([128, D], F32, name="f2br_sb")

        def _late_const_dmas():
            # Pool SWDGE queue: keeps the SP queue free for x prefetches
            nc.gpsimd.dma_start(qkb, qkb_d)
            nc.gpsimd.dma_start(vone6, vone6_d)
            nc.gpsimd.dma_start(vbr, vbr_d)
            nc.gpsimd.dma_start(wproj, wproj_d)
            nc.gpsimd.dma_start(pbr, pbr_d)
            nc.gpsimd.dma_start(wfc1, wfc1_d)
            nc.gpsimd.dma_start(g1b, g1b_d)
            nc.gpsimd.dma_start(wfc2, wfc2_d)
            nc.gpsimd.dma_start(f2br, f2br_d)

        def _body_phases():
            with ExitStack() as actx:
                # PSUM: mm(2) + tp(2) + sc(2x2) + av(2) = 8 banks
                pp = actx.enter_context(tc.tile_pool(name="pp", bufs=1,
                                                     space="PSUM"))
                st = actx.enter_context(tc.tile_pool(name="st", bufs=4))
                xpool = actx.enter_context(tc.tile_pool(name="xp", bufs=2))
                hpool = actx.enter_context(tc.tile_pool(name="hp", bufs=NT + 1))
                hTp = actx.enter_context(tc.tile_pool(name="hT", bufs=2))
                qkp = actx.enter_context(tc.tile_pool(name="qk", bufs=12))
                vp = actx.enter_context(tc.tile_pool(name="vp", bufs=2 * NT))
                tsbp = actx.enter_context(tc.tile_pool(name="tsb", bufs=2))
                polyp = actx.enter_context(tc.tile_pool(name="poly",
                                                        bufs=H * NT))
                aop = actx.enter_context(tc.tile_pool(name="ao", bufs=NT + 1))
                aTp = actx.enter_context(tc.tile_pool(name="aT", bufs=2))
                x2p = actx.enter_context(tc.tile_pool(name="x2p", bufs=GT))
                h2Tp = actx.enter_context(tc.tile_pool(name="h2T", bufs=1))
                gp = actx.enter_context(tc.tile_pool(name="gp",
                                                     bufs=FC_H + 1))
                outpl = actx.enter_context(tc.tile_pool(name="outl", bufs=1))

                x2_tiles = [None] * GT
                # h2T persists across batches: MLP chunks cross batch bounds
                h2T = h2Tp.tile([128, KC_D * TP], MT, name="h2T_all")
                h2T3 = h2T.rearrange("p (k c) -> p k c", c=TP)
                mlp_done = [0]  # chunks of 512 tokens emitted so far
                x_tiles = {}
                # v ring: 10 explicit tiles (2 sets of NT); the masked-ones
                # columns depend only on tt, so write them once up front
                # (AFTER the vone6 DMA is emitted — reads bind to prior writes)
                v_ring = []

                def _init_v_ring():
                    for s in range(2 * NT):
                        vt = vp.tile([128, H * VW], MT, tag=f"v{s}",
                                     name=f"v_{s}", bufs=1)
                        v_ring.append(vt)
                        nc.gpsimd.tensor_copy(
                            vt.rearrange("p (h c) -> p h c", c=VW)
                            [:, :, HD:HD + 2],
                            vone6[:, (s % NT) * H * 2:(s % NT + 1) * H * 2]
                            .rearrange("p (h c) -> p h c", c=2))

                def prefetch_x(b):
                    xa = xpool.tile([128, NT * D], F32, tag="x", name="x_all")
                    xa3 = xa.rearrange("p (t c) -> p t c", c=D)
                    src_v = xp[b * NP:(b + 1) * NP, :].rearrange(
                        "(t p) c -> p t c", p=128)
                    nc.sync.dma_start(xa3, src_v)
                    x_tiles[b] = [xa3[:, tt, :] for tt in range(NT)]

                def transpose3(src_t, dst3, dst_col, engine):
                    """Transpose [128(tok), 384(feat)] -> dst3[:, k, col:+128].
                    "act"/"vec": PE-transpose 3 chunks into one [128,384] psum
                    + ONE strided 3-way copy.  "dma": DMA crossbar (latency-
                    tolerant consumers only), issued on the ACT queue."""
                    if engine == "dma":
                        nc.sync.dma_start_transpose(
                            dst3[:, :, dst_col:dst_col + 128], src_t)
                        return
                    tp_ps = pp.tile([128, KC_D * 128], src_t.dtype, tag="fr",
                                    name="tp_ps", space="PSUM", bufs=2)
                    for kc in range(KC_D):
                        nc.tensor.transpose(
                            tp_ps[:, kc * 128:(kc + 1) * 128],
                            src_t[:, kc * 128:(kc + 1) * 128], ident)
                    tp3 = tp_ps.rearrange("p (k c) -> p k c", c=128)
                    dview = dst3[:, :, dst_col:dst_col + 128]
                    if engine == "act":
                        nc.scalar.activation(dview, tp3, AF.Copy)
                    else:
                        nc.vector.tensor_copy(dview, tp3)

                def attention(b, fill):
                    if b + 1 < BPC:
                        prefetch_x(b + 1)
                    # --- LN1 + transpose to feature-major hT ---
                    hT = hTp.tile([128, KC_D * NP], MT, tag="hT",
                                  name=f"hT{b}")
                    hT3 = hT.rearrange("p (k c) -> p k c", c=NP)
                    x_ts = x_tiles.pop(b)
                    h_ts = []
                    with tc.high_priority(1200):
                        for tt in range(NT):
                            h_t = hpool.tile([128, D], MT, tag="h", name="h_t")
                            h_ts.append(h_t)
                            _ln(nc, st, consts, x_ts[tt], h_t, "1")
                    fill(3)
                    with tc.high_priority(1200):
                        for tt in range(NT):
                            transpose3(h_ts[tt], hT3, tt * 128, "act")

                    # --- QKV (q,k feature-major; fc order lets hp0 start early)
                    qk = [None] * 6
                    for fc in (0, 3, 1, 4, 2, 5):
                        t = qkp.tile([128, NP], MT, tag="qk", name=f"qk{b}_{fc}")
                        qk[fc] = t
                        for c0, c1 in ((0, 512), (512, NP)):
                            ps = pp.tile([128, 512], F32, tag="fr", name="qk_ps",
                                         space="PSUM", bufs=2)
                            for kc in range(KC_D):
                                nc.tensor.matmul(
                                    ps[:, 0:c1 - c0],
                                    wqkv[:, kc * 3 * D + fc * 128:
                                         kc * 3 * D + fc * 128 + 128],
                                    hT3[:, kc, c0:c1],
                                    start=(kc == 0), stop=(kc == KC_D - 1),
                                )
                            if add_qkb:
                                nc.vector.tensor_scalar(
                                    out=t[:, c0:c1], in0=ps[:, 0:c1 - c0],
                                    scalar1=qkb[:, fc:fc + 1], scalar2=None,
                                    op0=ALU.add)
                            else:
                                nc.vector.tensor_copy(t[:, c0:c1],
                                                      ps[:, 0:c1 - c0])

                    fill(1)
                    # --- v token-major [128, H*VW]: [v | masked-ones | pad]
                    v_sb = []
                    for tt in range(NT):
                        ps = pp.tile([128, 512], F32, tag="fr", name="v_ps",
                                     space="PSUM", bufs=2)
                        for kc in range(KC_D):
                            nc.tensor.matmul(
                                ps[:, 0:D],
                                hT3[:, kc, tt * 128:(tt + 1) * 128],
                                wqkv[:, kc * 3 * D + 768:kc * 3 * D + 1152],
                                start=(kc == 0), stop=(kc == KC_D - 1),
                            )
                        vt = v_ring[(b % 2) * NT + tt]
                        v_sb.append(vt)
                        v3 = vt.rearrange("p (h c) -> p h c", c=VW)
                        ps3 = ps[:, 0:D].rearrange("p (h c) -> p h c", c=HD)
                        if add_vb:
                            nc.vector.tensor_add(
                                v3[:, :, 0:HD], ps3,
                                vbr.rearrange("p (h c) -> p h c", c=HD))
                            if tt == NT - 1:
                                nc.vector.tensor_scalar(
                                    out=v3[:, :, 0:HD], in0=v3[:, :, 0:HD],
                                    scalar1=vone6[:, tt * H * 2:tt * H * 2 + 1],
                                    scalar2=None, op0=ALU.mult)
                        else:
                            # padded-token rows of psum are exactly 0 (h_pad==0)
                            nc.vector.tensor_copy(v3[:, :, 0:HD], ps3)

                    fill(1)
                    # --- scores + poly: all 6 heads (pair-interleaved PE rows)
                    # high priority: the ACT square stretch paces the whole
                    # batch; score matmuls must preempt interleaved MLP work
                    polys = {}
                    for hp in range(H // 2):
                        h0, h1 = 2 * hp, 2 * hp + 1
                        for h in (h0, h1):
                            for mc in range(NT):
                                polys[(h, mc)] = polyp.tile(
                                    [128, NP], MT, tag="poly",
                                    name=f"poly{h}_{mc}")
                                if SIMSAFE:
                                    # pad cols are never consumed (masked via
                                    # v rows); init only for CoreSim checks
                                    nc.vector.memset(
                                        polys[(h, mc)][:, NV:NP], 0.0)
                        prio_ctx = tc.high_priority(1500)
                        prio_ctx.__enter__()
                        for mc in range(NT):
                            pss = {}
                            for h in (h0, h1):
                                fcq = h // 2
                                row = (h % 2) * 64
                                ps = pp.tile([128, 1024], F32, tag="sc",
                                             name="sc_ps", space="PSUM",
                                             bufs=2)
                                pss[h] = ps
                                for c0, c1 in ((0, 512), (512, NV)):
                                    nc.tensor.matmul(
                                        ps[:, c0:c1],
                                        qk[3 + fcq][row:row + 64,
                                                    mc * 128:(mc + 1) * 128],
                                        qk[fcq][row:row + 64, c0:c1],
                                        start=True, stop=True,
                                    )
                            for h in (h0, h1):
                                if polymode == "lin":
                                    nc.scalar.activation(
                                        polys[(h, mc)][:, 0:NV],
                                        pss[h][:, 0:NV], AF.Relu,
                                        scale=pB, bias=c_pc2)
                                elif polymode == "relu":
                                    tsb = tsbp.tile([128, NP], BF16, tag="tsb",
                                                    name="t_sb")
                                    nc.scalar.activation(
                                        tsb[:, 0:NV], pss[h][:, 0:NV],
                                        AF.Square, bias=c_b2a)
                                    nc.scalar.activation(
                                        polys[(h, mc)][:, 0:NV],
                                        tsb[:, 0:NV], AF.Relu,
                                        scale=pA, bias=c_pc2)
                                else:  # fold
                                    tsb = tsbp.tile([128, NP], BF16, tag="tsb",
                                                    name="t_sb")
                                    nc.scalar.activation(
                                        tsb[:, 0:NV], pss[h][:, 0:NV],
                                        AF.Square, bias=c_b2a)
                                    # DVE 4x mode: all-SBUF, 2-byte, packed
                                    nc.vector.tensor_scalar(
                                        out=polys[(h, mc)][:, 0:NV],
                                        in0=tsb[:, 0:NV], scalar1=C2A,
                                        scalar2=0.0, op0=ALU.add, op1=ALU.max)
                        prio_ctx.__exit__(None, None, None)
                        fill(2)

                    # --- attn @ [v | ones]: all heads in one [128,396] psum
                    ao_t = []
                    for nt in range(NT):
                        ps = pp.tile([128, H * VW], F32, tag="mm",
                                     name="av_ps", space="PSUM", bufs=2)
                        for h in range(H):
                            off = h * VW
                            for mc in range(NT):
                                nc.tensor.matmul(
                                    ps[:, off:off + VW],
                                    polys[(h, mc)][:, nt * 128:(nt + 1) * 128],
                                    v_sb[mc][:, off:off + VW],
                                    start=(mc == 0), stop=(mc == NT - 1),
                                )
                        # one copy frees the psum bank; normalize from SBUF
                        avs = st.tile([128, H * VW], F32, tag="avs", name="avs",
                                      bufs=2)
                        nc.vector.tensor_copy(avs, ps)
                        zv = avs.rearrange("p (h c) -> p h c", c=VW)[:, :, HD]
                        zt = st.tile([128, H], F32, tag="zt", name="zt")
                        nc.vector.tensor_scalar_add(zt, zv, float(sc["eps_eff"]))
                        rz = st.tile([128, H], F32, tag="rz", name="rz")
                        nc.vector.reciprocal(rz, zt)
                        ao = aop.tile([128, D], MT, tag="ao", name=f"ao{b}_{nt}")
                        ao_t.append(ao)
                        for h in range(H):
                            eng = nc.vector if h < 3 else nc.gpsimd
                            eng.tensor_scalar(
                                out=ao[:, h * HD:(h + 1) * HD],
                                in0=avs[:, h * VW:h * VW + HD],
                                scalar1=rz[:, h:h + 1], scalar2=None,
                                op0=ALU.mult)

                    # --- transpose attn_out, proj, residual, LN2 (fused) ---
                    aT = aTp.tile([128, KC_D * NP], MT, tag="aT", name=f"aT{b}")
                    aT3 = aT.rearrange("p (k c) -> p k c", c=NP)
                    for nt in range(NT):
                        transpose3(ao_t[nt], aT3, nt * 128, "act")
                    for tt in range(NT):
                        gt = b * NT + tt
                        ps = pp.tile([128, 512], F32, tag="mm", name="pj_ps",
                                     space="PSUM", bufs=2)
                        for kc in range(KC_D):
                            nc.tensor.matmul(
                                ps[:, 0:D],
                                aT3[:, kc, tt * 128:(tt + 1) * 128],
                                wproj[:, kc * D:(kc + 1) * D],
                                start=(kc == 0), stop=(kc == KC_D - 1),
                            )
                        x2t = x2p.tile([128, D], F32, tag="x2", name=f"x2_{gt}")
                        x2_tiles[gt] = x2t
                        if add_pb:
                            nc.vector.tensor_add(x2t, ps[:, 0:D], pbr)
                            nc.vector.tensor_add(x2t, x2t, x_ts[tt])
                        else:
                            nc.vector.tensor_add(x2t, ps[:, 0:D], x_ts[tt])
                        # LN2 per tile right after its residual: the h2T
                        # transpose is in flight while the next proj runs
                        h2_t = hpool.tile([128, D], MT, tag="h2", name="h2_t")
                        _ln(nc, st, consts, x2t, h2_t, "2")
                        transpose3(h2_t, h2T3, gt * 128, "act")

                def mlp_chunk(nch):
                    """Generator: yields after each piece so the caller can
                    interleave MLP work into the next batch's attention."""
                    g_sb = []
                    for fc in range(FC_H):
                        ps = pp.tile([128, 512], F32, tag="mm", name="f1_ps",
                                     space="PSUM", bufs=2)
                        for kc in range(KC_D):
                            nc.tensor.matmul(
                                ps,
                                wfc1[:, kc * HID + fc * 128:
                                     kc * HID + fc * 128 + 128],
                                h2T3[:, kc, nch * 512:(nch + 1) * 512],
                                start=(kc == 0), stop=(kc == KC_D - 1),
                            )
                        gt_sb = gp.tile([128, 512], MT, tag="g",
                                        name=f"g{nch}_{fc}")
                        g_sb.append(gt_sb)
                        nc.scalar.activation(
                            gt_sb, ps,
                            AF.Square if gelmode == "quad" else AF.Identity,
                            bias=g1b[:, fc:fc + 1])
                        if fc % 3 == 2:
                            yield
                    ota = outpl.tile([128, 4 * D], F32, tag="ot", name="out_a")
                    ota3 = ota.rearrange("p (t c) -> p t c", c=D)
                    for tt in range(4):
                        gt = nch * 4 + tt
                        ps = pp.tile([128, 512], F32, tag="mm", name="f2_ps",
                                     space="PSUM", bufs=2)
                        for fc in range(FC_H):
                            nc.tensor.matmul(
                                ps[:, 0:D],
                                g_sb[fc][:, tt * 128:(tt + 1) * 128],
                                wfc2[:, fc * D:(fc + 1) * D],
                                start=(fc == 0), stop=(fc == FC_H - 1),
                            )
                        ot = ota3[:, tt, :]
                        if add_f2b:
                            nc.vector.tensor_add(ot, ps[:, 0:D], f2br)
                            nc.vector.tensor_add(ot, ot, x2_tiles[gt])
                        else:
                            nc.vector.tensor_add(ot, ps[:, 0:D], x2_tiles[gt])
                        yield
                    # out DMA on the Pool SWDGE queue: SP head-of-line
                    # blocking would park x-prefetch/h2T behind this wait
                    if bench_R:
                        nc.gpsimd.dma_start(outp[0:128, :], ota3[:, 0, :])
                    else:
                        dst_v = outp[nch * 512:(nch + 1) * 512, :].rearrange(
                            "(t p) c -> p t c", p=128)
                        nc.gpsimd.dma_start(dst_v, ota3)

                prefetch_x(0)
                _late_const_dmas()
                _init_v_ring()
                pending = []  # MLP piece generators ready for interleaving

                def fill(n):
                    for _ in range(n):
                        while pending:
                            if next(pending[0], StopIteration) is StopIteration:
                                pending.pop(0)
                                continue
                            break
                        else:
                            return

                for b in range(BPC):
                    attention(b, fill)
                    fill(10 ** 6)  # drain leftovers
                    ready = ((b + 1) * NP) // 512
                    while mlp_done[0] < ready:
                        pending.append(mlp_chunk(mlp_done[0]))
                        mlp_done[0] += 1
                fill(10 ** 6)

        if bench_R:
            with tc.For_i(0, bench_R, 1):
                _body_phases()
        else:
            _body_phases()

    nc.compile()
    return nc


def host_prep(inputs):
    """Fold LN affine params into weights, build per-core input maps."""
    f = lambda k: np.asarray(inputs[k], dtype=np.float32)
    x = f("x")
    qkv_w, qkv_b = f("qkv_w"), f("qkv_b")
    proj_w, proj_b = f("proj_w"), f("proj_b")
    fc1_w, fc1_b = f("fc1_w"), f("fc1_b")
    fc2_w, fc2_b = f("fc2_w"), f("fc2_b")
    ln1_g, ln1_b = f("ln1_g"), f("ln1_b")
    ln2_g, ln2_b = f("ln2_g"), f("ln2_b")
    attn_abc = f("attn_abc")
    gelu_abc = f("gelu_abc")

    scale = HD ** -0.5
    pA = float(attn_abc[0]) * scale * scale
    pB = float(attn_abc[1]) * scale
    pC = float(attn_abc[2])
    sc = {"pA": pA, "pB": pB, "pC": pC}
    if pA > 1e-12:
        sc["polymode"] = "fold"
        sc["B2A"] = pB / (2.0 * pA)
        sc["pC2"] = pC - pB * pB / (4.0 * pA)
        sc["C2A"] = sc["pC2"] / pA
        eps_eff = ATTN_EPS / pA
    elif pA < -1e-12:
        sc["polymode"] = "relu"
        sc["B2A"] = pB / (2.0 * pA)
        sc["pC2"] = pC - pB * pB / (4.0 * pA)
        sc["C2A"] = 0.0
        eps_eff = ATTN_EPS
    else:
        sc["polymode"] = "lin"
        sc["B2A"] = 0.0
        sc["pC2"] = pC
        sc["C2A"] = 0.0
        eps_eff = ATTN_EPS

    ga, gb, gc = float(gelu_abc[0]), float(gelu_abc[1]), float(gelu_abc[2])
    fc1_b_eff = (fc1_b + ln2_b @ fc1_w).astype(np.float32)
    if abs(ga) > 1e-12:
        # gelu(u) = ga*(u + gb/2ga)^2 + (gc - gb^2/4ga); fold ga into fc2_w
        # and the constant into fc2_b via column sums.
        sc["gelmode"] = "quad"
        gC2 = gc - gb * gb / (4.0 * ga)
        g1bias = fc1_b_eff + gb / (2.0 * ga)
        fc2_w_eff = ga * fc2_w
        fc2_b_eff = fc2_b + gC2 * fc2_w.sum(axis=0)
    else:
        # gelu(u) = gb*u + gc
        sc["gelmode"] = "lin"
        g1bias = fc1_b_eff
        fc2_w_eff = gb * fc2_w
        fc2_b_eff = fc2_b + gc * fc2_w.sum(axis=0)

    sc["eps_eff"] = float(eps_eff)
    qkv_w_eff = (ln1_g[:, None] * qkv_w).astype(np.float32)
    qkv_b_eff = (qkv_b + ln1_b @ qkv_w).astype(np.float32)
    fc1_w_eff = (ln2_g[:, None] * fc1_w).astype(np.float32)

    sc["add_vb"] = bool(np.any(qkv_b_eff[2 * D:] != 0.0))
    sc["add_pb"] = bool(np.any(proj_b != 0.0))
    sc["add_f2b"] = bool(np.any(fc2_b_eff != 0.0))
    sc["add_qkb"] = bool(np.any(qkv_b_eff[:2 * D] != 0.0))

    common = {
        "wqkv": np.ascontiguousarray(
            qkv_w_eff.reshape(KC_D, 128, 3 * D).transpose(1, 0, 2)
            .reshape(128, KC_D * 3 * D)).astype(NPBF),
        "wproj": np.ascontiguousarray(
            proj_w.reshape(KC_D, 128, D).transpose(1, 0, 2)
            .reshape(128, KC_D * D)).astype(NPBF),
        "wfc1": np.ascontiguousarray(
            fc1_w_eff.reshape(KC_D, 128, HID).transpose(1, 0, 2)
            .reshape(128, KC_D * HID)).astype(NPBF),
        "wfc2": np.ascontiguousarray(
            fc2_w_eff.astype(np.float32).reshape(FC_H, 128, D).transpose(1, 0, 2)
            .reshape(128, FC_H * D)).astype(NPBF),
        "qkb": np.ascontiguousarray(qkv_b_eff[:2 * D].reshape(6, 128).T),
        "vbr": np.ascontiguousarray(
            np.broadcast_to(qkv_b_eff[2 * D:], (128, D))),
        "pbr": np.ascontiguousarray(np.broadcast_to(proj_b, (128, D))),
        "f2br": np.ascontiguousarray(np.broadcast_to(fc2_b_eff, (128, D))),
        "g1b": np.ascontiguousarray(g1bias.reshape(FC_H, 128).T),
        "vone6": None,
        "ident": np.eye(128, dtype=np.float32).astype(NPBF),
    }
    mask = (np.arange(NP) < N).astype(np.float32)  # [640]
    mz = np.zeros((NT, 128, H, 2), np.float32)
    mz[:, :, :, 0] = mask.reshape(NT, 128)[:, :, None]
    common["vone6"] = np.ascontiguousarray(
        mz.transpose(1, 0, 2, 3).reshape(128, NT * H * 2))
    common = {k: (np.ascontiguousarray(v, dtype=np.float32)
                  if v.dtype != NPBF else v)
              for k, v in common.items()}

    in_maps = []
    for c in range(NCORES):
        xp_c = np.zeros((BPC, NP, D), np.float32)
        xp_c[:, :N, :] = x[c * BPC:(c + 1) * BPC]
        m = dict(common)
        m["xp"] = xp_c.reshape(TP, D)
        in_maps.append(m)
    return sc, in_maps


_CACHE = {}


def _get_program(sc):
    key = tuple(sorted((k, v) for k, v in sc.items()))
    if key not in _CACHE:
        _CACHE[key] = build_program(sc)
    return _CACHE[key]


def _runner_meta(nc):
    partition_name = nc.partition_id_tensor.name if nc.partition_id_tensor else None
    in_names, out_names, out_avals, zero_outs = [], [], [], []
    import jax
    for alloc in nc.m.functions[0].allocations:
        if not isinstance(alloc, mybir.MemoryLocationSet):
            continue
        name = alloc.memorylocations[0].name
        if alloc.kind == "ExternalInput":
            if name != partition_name:
                in_names.append(name)
        elif alloc.kind == "ExternalOutput":
            out_names.append(name)
            shape = tuple(alloc.tensor_shape)
            dtype = mybir.dt.np(alloc.dtype)
            out_avals.append(jax.core.ShapedArray(shape, dtype))
            zero_outs.append(np.zeros(shape, dtype))
    return partition_name, in_names, out_names, out_avals, zero_outs


_RUNNERS = {}


def _make_runner(nc, chain):
    """Jitted 8-core runner executing the NEFF `chain` times back-to-back
    (iteration i+1 consumes iteration i's outputs as its scratch buffers,
    forcing sequential execution)."""
    key = (id(nc), chain)
    if key in _RUNNERS:
        return _RUNNERS[key]
    import jax
    from jax.sharding import Mesh, PartitionSpec, NamedSharding
    from jax.experimental.shard_map import shard_map
    from concourse.bass2jax import (_bass_exec_p, install_neuronx_cc_hook,
                                    partition_id_tensor)
    install_neuronx_cc_hook()
    partition_name, in_names, out_names, out_avals, zero_outs = _runner_meta(nc)
    n_params = len(in_names)
    all_in = list(in_names) + list(out_names)
    if partition_name is not None:
        all_in = all_in + [partition_name]

    def _body(*args):
        ins = list(args[:n_params])
        cur = list(args[n_params:])
        for _ in range(chain):
            operands = ins + cur
            if partition_name is not None:
                operands = operands + [partition_id_tensor()]
            cur = list(_bass_exec_p.bind(
                *operands,
                out_avals=tuple(out_avals),
                in_names=tuple(all_in),
                out_names=tuple(out_names),
                lowering_input_output_aliases=(),
                sim_require_finite=True,
                sim_require_nnan=True,
                nc=nc,
            ))
        return tuple(cur)

    devices = jax.devices()[:NCORES]
    mesh = Mesh(np.asarray(devices), ("core",))
    nin = n_params + len(out_names)
    sharded = jax.jit(
        shard_map(_body, mesh=mesh,
                  in_specs=(PartitionSpec("core"),) * nin,
                  out_specs=(PartitionSpec("core"),) * len(out_names),
                  check_rep=False),
        keep_unused=True)
    shard = NamedSharding(mesh, PartitionSpec("core"))
    r = (sharded, shard, in_names, out_names, zero_outs)
    _RUNNERS[key] = r
    return r


def _concat_inputs(in_maps, in_names, zero_outs):
    concat_in = [np.concatenate([np.asarray(in_maps[c][n]) for c in range(NCORES)],
                                axis=0) for n in in_names]
    concat_zero = [np.zeros((NCORES * z.shape[0], *z.shape[1:]), z.dtype)
                   for z in zero_outs]
    return concat_in, concat_zero


def kernel(**inputs):
    sc, in_maps = host_prep(inputs)
    nc = _get_program(sc)
    sharded, shard, in_names, out_names, zero_outs = _make_runner(nc, 1)
    concat_in, concat_zero = _concat_inputs(in_maps, in_names, zero_outs)
    out_arrs = sharded(*concat_in, *concat_zero)
    oi = out_names.index("outp")
    full = np.asarray(out_arrs[oi]).reshape(NCORES, BPC, NP, D)[:, :, :N, :]
    return np.ascontiguousarray(full.reshape(B, N, D), dtype=np.float32)


def bench(inputs, chain=65, reps=15):
    """Measure per-execution HW time: bench-variant programs with internal
    (unfed) inputs and an in-program For_i repeat loop; difference R=chain
    vs R=1 wall time to cancel dispatch/transfer overhead."""
    import time
    import jax
    sc, in_maps = host_prep(inputs)

    def _run_R(R):
        key = (tuple(sorted((k, v) for k, v in sc.items())), "bench", R)
        if key not in _CACHE:
            _CACHE[key] = build_program(sc, bench_R=R)
        nc = _CACHE[key]
        sharded, shard, in_names, out_names, zero_outs = _make_runner(nc, 1)
        concat_in, concat_zero = _concat_inputs(
            [dict() for _ in range(NCORES)], in_names, zero_outs)
        out = sharded(*concat_in, *concat_zero)
        jax.block_until_ready(out)
        ts = []
        for _ in range(reps):
            t0 = time.perf_counter()
            out = sharded(*concat_in, *concat_zero)
            jax.block_until_ready(out)
            ts.append(time.perf_counter() - t0)
        return min(ts)

    t1 = _run_R(1)
    tn = _run_R(chain)
    per_exec_ns = (tn - t1) / (chain - 1) * 1e9
    return per_exec_ns, t1, tn


# revision 60
# speedup vs baseline: 1.3121x; 1.0169x over previous
"""Trainium2 Bass kernel: ViT-style transformer block with polynomial attention.

Sharding: pure data-parallel over batch B=32 across 8 NeuronCores (4 batch
elements per core).  No collectives.  Each core computes the full block for
its batch slice; host gathers/concats.

v2 layout strategy (single fused pipeline, per-batch interleaved):
  - tokens padded per-batch 577 -> 640 (5 tiles of 128); 4*640 = 2560/core.
  - attention(b) and the MLP chunks that become ready after batch b are
    emitted in one program-order stream so every engine (esp. PE) stays
    continuously busy: while PE runs MLP(b) matmuls, DVE/ACT run LN/poly
    prep for attention(b+1).
  - LN gains/biases folded into downstream weights on host; on-chip LN is
    bn_stats/bn_aggr (DVE) + Rsqrt (ACT, one op) + mu*rstd (Pool) + one
    fused scale/shift (DVE, 2x mode).
  - transposes: PE-transpose 3x[128,128] chunks into ONE [128,384] PSUM
    tile, then ONE strided 3-way copy to SBUF (halves per-copy overhead).
  - scores per (head, m-chunk) go into a [128,1024] two-bank PSUM tile so
    the PolyAttn Square (+B/2A bias) is ONE ACT op over 578 cols; the fold
    relu(sq + C2A) runs on DVE in 4x bf16 mode (all-SBUF, 2-byte).
  - attn@v accumulates ALL 6 heads into one [128,396] PSUM tile per token
    tile ([64 v | masked-ones | pad] per head), so Z extraction is one
    strided op + one reciprocal; 6 per-head scales produce attn-out.
  - PolyGELU is a pure Square on ACT (scale folded into fc2 weights, the
    constant into fc2 bias via column sums).
  - all matmul operands bf16; fp32 accum in PSUM; residual stream fp32.
"""

import sys

for _p in ("/opt/trn_rl_repo",):
    if _p not in sys.path:
        sys.path.insert(0, _p)

from contextlib import ExitStack

import os

import numpy as np
import ml_dtypes

SIMSAFE = bool(int(os.environ.get("K_SIMSAFE", "0")))

import concourse.bacc as bacc
import concourse.mybir as mybir
import concourse.tile as tile

B, N, D, H = 32, 577, 384, 6
HD = D // H            # 64
HID = 4 * D            # 1536
LN_EPS = 1e-5
ATTN_EPS = 1e-6

NCORES = 8
BPC = B // NCORES      # 4 batches per core
NP = 640               # padded tokens per batch (5 * 128)
NT = NP // 128         # 5 token tiles per batch
TP = BPC * NP          # 2560 tokens per core
GT = TP // 128         # 20 token tiles per core
KC_D = D // 128        # 3 contraction chunks over D
FC_H = HID // 128      # 12 chunks over hidden
NV = N + 1             # 578: even score/poly width covering valid n tokens
VW = HD + 2            # 66: per-head v width ([v | masked-ones | pad])

F32 = mybir.dt.float32
BF16 = mybir.dt.bfloat16
AF = mybir.ActivationFunctionType
ALU = mybir.AluOpType

MT = BF16              # matmul operand dtype
FP8 = mybir.dt.float8e4   # q/k score operands (DoubleRow perf mode)
NPBF = np.dtype(ml_dtypes.bfloat16)


def _ln(nc, st, consts, x_t, out_t, tg):
    """LayerNorm center+scale (gain/bias folded into downstream weights).
    Mean/var in one DVE pass; h = (x - mu) * rstd in one fused op.  Stat
    tile tags are per-callsite (tg) so LN1(b+1) is not ring-serialized
    behind LN2(b)."""
    s6 = st.tile([128, 6], F32, tag="s6" + tg, name="s6")
    nc.vector.bn_stats(s6, x_t)
    mv = st.tile([128, 2], F32, tag="mv" + tg, name="mv")
    nc.vector.bn_aggr(mv, s6)
    sd = st.tile([128, 1], F32, tag="sd" + tg, name="sd")
    nc.scalar.activation(sd, mv[:, 1:2], AF.Sqrt, bias=consts["lneps"])
    rstd = st.tile([128, 1], F32, tag="rstd" + tg, name="rstd")
    nc.vector.reciprocal(rstd, sd)
    nc.vector.tensor_scalar(out=out_t, in0=x_t, scalar1=mv[:, 0:1],
                            scalar2=rstd, op0=ALU.subtract, op1=ALU.mult)


def build_program(sc, bench_R=0):
    """sc: dict of host scalar constants / flags."""
    nc = bacc.Bacc("TRN2", target_bir_lowering=False, debug=False)

    kind_in = "Internal" if bench_R else "ExternalInput"
    xp = nc.dram_tensor("xp", [TP, D], F32, kind=kind_in).ap()
    wqkv_d = nc.dram_tensor("wqkv", [128, KC_D * 3 * D], MT, kind=kind_in).ap()
    wproj_d = nc.dram_tensor("wproj", [128, KC_D * D], MT, kind=kind_in).ap()
    wfc1_d = nc.dram_tensor("wfc1", [128, KC_D * HID], MT, kind=kind_in).ap()
    wfc2_d = nc.dram_tensor("wfc2", [128, FC_H * D], MT, kind=kind_in).ap()
    qkb_d = nc.dram_tensor("qkb", [128, 6], F32, kind=kind_in).ap()
    vbr_d = nc.dram_tensor("vbr", [128, D], F32, kind=kind_in).ap()
    pbr_d = nc.dram_tensor("pbr", [128, D], F32, kind=kind_in).ap()
    f2br_d = nc.dram_tensor("f2br", [128, D], F32, kind=kind_in).ap()
    g1b_d = nc.dram_tensor("g1b", [128, FC_H], F32, kind=kind_in).ap()
    vone6_d = nc.dram_tensor("vone6", [128, NT * H * 2], F32, kind=kind_in).ap()
    ident_d = nc.dram_tensor("ident", [128, 128], MT, kind=kind_in).ap()
    outp = nc.dram_tensor("outp", [128 if bench_R else TP, D], F32,
                          kind="ExternalOutput").ap()

    A2 = sc["B2A"]          # B/(2A): square-pass bias
    C2A = sc["C2A"]         # (C - B^2/(4A))/A: fold add before max(.,0)
    polymode = sc["polymode"]  # "fold" (A>0), "relu" (A<0), "lin" (A==0)
    pC2 = sc["pC2"]
    pA = sc["pA"]
    pB = sc["pB"]
    pC = sc["pC"]
    gelmode = sc["gelmode"]
    add_vb = sc["add_vb"]
    add_pb = sc["add_pb"]
    add_f2b = sc["add_f2b"]
    add_qkb = sc.get("add_qkb", True)

    with ExitStack() as octx:
        tc = octx.enter_context(tile.TileContext(nc))
        cp = octx.enter_context(tc.tile_pool(name="consts", bufs=1))
        # persistent constants — attention-phase tensors first so the PE can
        # start as soon as possible; MLP weights arrive much later.
        # attn-critical consts on the ACT queue (short; LN's Sqrt shares it)
        ident = cp.tile([128, 128], MT, name="ident_sb")
        nc.scalar.dma_start(ident, ident_d)
        wqkv = cp.tile([128, KC_D * 3 * D], MT, name="wqkv_sb")
        for kc in range(KC_D):
            nc.scalar.dma_start(wqkv[:, kc * 3 * D:(kc + 1) * 3 * D],
                                wqkv_d[:, kc * 3 * D:(kc + 1) * 3 * D])
        c_lneps = cp.tile([128, 1], F32, name="c_lneps")
        nc.vector.memset(c_lneps, LN_EPS)
        c_b2a = cp.tile([128, 1], F32, name="c_b2a")
        nc.vector.memset(c_b2a, A2)
        c_pc2 = cp.tile([128, 1], F32, name="c_pc2")
        nc.vector.memset(c_pc2, pC2 if polymode == "relu" else pC)
        consts = {"lneps": c_lneps}
        # the rest goes on the SP queue, issued AFTER batch 0's x prefetch
        # (see _body_phases) so LN1 isn't starved at startup.
        qkb = cp.tile([128, 6], F32, name="qkb_sb")
        vone6 = cp.tile([128, NT * H * 2], F32, name="vone6_sb")
        vbr = cp.tile([128, D], F32, name="vbr_sb")
        wproj = cp.tile([128, KC_D * D], MT, name="wproj_sb")
        pbr = cp.tile([128, D], F32, name="pbr_sb")
        wfc1 = cp.tile([128, KC_D * HID], MT, name="wfc1_sb")
        g1b = cp.tile([128, FC_H], F32, name="g1b_sb")
        wfc2 = cp.tile([128, FC_H * D], MT, name="wfc2_sb")
        f2br = cp.tile([128, D], F32, name="f2br_sb")

        def _late_const_dmas():
            # Pool SWDGE queue: keeps the SP queue free for x prefetches
            nc.gpsimd.dma_start(qkb, qkb_d)
            nc.gpsimd.dma_start(vone6, vone6_d)
            nc.gpsimd.dma_start(vbr, vbr_d)
            nc.gpsimd.dma_start(wproj, wproj_d)
            nc.gpsimd.dma_start(pbr, pbr_d)
            nc.gpsimd.dma_start(wfc1, wfc1_d)
            nc.gpsimd.dma_start(g1b, g1b_d)
            nc.gpsimd.dma_start(wfc2, wfc2_d)
            nc.gpsimd.dma_start(f2br, f2br_d)

        def _body_phases():
            with ExitStack() as actx:
                # PSUM: mm(2) + tp(2) + sc(2x2) + av(2) = 8 banks
                pp = actx.enter_context(tc.tile_pool(name="pp", bufs=1,
                                                     space="PSUM"))
                st = actx.enter_context(tc.tile_pool(name="st", bufs=4))
                xpool = actx.enter_context(tc.tile_pool(name="xp", bufs=2))
                hpool = actx.enter_context(tc.tile_pool(name="hp", bufs=NT + 1))
                hTp = actx.enter_context(tc.tile_pool(name="hT", bufs=2))
                qkp = actx.enter_context(tc.tile_pool(name="qk", bufs=12))
                vp = actx.enter_context(tc.tile_pool(name="vp", bufs=2 * NT))
                tsbp = actx.enter_context(tc.tile_pool(name="tsb", bufs=2))
                polyp = actx.enter_context(tc.tile_pool(name="poly",
                                                        bufs=H * NT))
                aop = actx.enter_context(tc.tile_pool(name="ao", bufs=NT + 1))
                aTp = actx.enter_context(tc.tile_pool(name="aT", bufs=2))
                x2p = actx.enter_context(tc.tile_pool(name="x2p", bufs=GT))
                h2Tp = actx.enter_context(tc.tile_pool(name="h2T", bufs=1))
                gp = actx.enter_context(tc.tile_pool(name="gp",
                                                     bufs=FC_H + 1))
                outpl = actx.enter_context(tc.tile_pool(name="outl", bufs=1))

                x2_tiles = [None] * GT
                # h2T persists across batches: MLP chunks cross batch bounds
                h2T = h2Tp.tile([128, KC_D * TP], MT, name="h2T_all")
                h2T3 = h2T.rearrange("p (k c) -> p k c", c=TP)
                mlp_done = [0]  # chunks of 512 tokens emitted so far
                x_tiles = {}
                # v ring: 10 explicit tiles (2 sets of NT); the masked-ones
                # columns depend only on tt, so write them once up front
                # (AFTER the vone6 DMA is emitted — reads bind to prior writes)
                v_ring = []

                def _init_v_ring():
                    for s in range(2 * NT):
                        vt = vp.tile([128, H * VW], MT, tag=f"v{s}",
                                     name=f"v_{s}", bufs=1)
                        v_ring.append(vt)
                        nc.gpsimd.tensor_copy(
                            vt.rearrange("p (h c) -> p h c", c=VW)
                            [:, :, HD:HD + 2],
                            vone6[:, (s % NT) * H * 2:(s % NT + 1) * H * 2]
                            .rearrange("p (h c) -> p h c", c=2))

                def prefetch_x(b):
                    xa = xpool.tile([128, NT * D], F32, tag="x", name="x_all")
                    xa3 = xa.rearrange("p (t c) -> p t c", c=D)
                    src_v = xp[b * NP:(b + 1) * NP, :].rearrange(
                        "(t p) c -> p t c", p=128)
                    nc.sync.dma_start(xa3, src_v)
                    x_tiles[b] = [xa3[:, tt, :] for tt in range(NT)]

                def transpose3(src_t, dst3, dst_col, engine):
                    """Transpose [128(tok), 384(feat)] -> dst3[:, k, col:+128].
                    "act"/"vec": PE-transpose 3 chunks into one [128,384] psum
                    + ONE strided 3-way copy.  "dma": DMA crossbar (latency-
                    tolerant consumers only), issued on the ACT queue."""
                    if engine == "dma":
                        nc.sync.dma_start_transpose(
                            dst3[:, :, dst_col:dst_col + 128], src_t)
                        return
                    tp_ps = pp.tile([128, KC_D * 128], src_t.dtype, tag="fr",
                                    name="tp_ps", space="PSUM", bufs=2)
                    for kc in range(KC_D):
                        nc.tensor.transpose(
                            tp_ps[:, kc * 128:(kc + 1) * 128],
                            src_t[:, kc * 128:(kc + 1) * 128], ident)
                    tp3 = tp_ps.rearrange("p (k c) -> p k c", c=128)
                    dview = dst3[:, :, dst_col:dst_col + 128]
                    if engine == "act":
                        nc.scalar.activation(dview, tp3, AF.Copy)
                    else:
                        nc.vector.tensor_copy(dview, tp3)

                def attention(b, fill):
                    if b + 1 < BPC:
                        prefetch_x(b + 1)
                    # --- LN1 + transpose to feature-major hT ---
                    hT = hTp.tile([128, KC_D * NP], MT, tag="hT",
                                  name=f"hT{b}")
                    hT3 = hT.rearrange("p (k c) -> p k c", c=NP)
                    x_ts = x_tiles.pop(b)
                    h_ts = []
                    with tc.high_priority(1200):
                        for tt in range(NT):
                            h_t = hpool.tile([128, D], MT, tag="h", name="h_t")
                            h_ts.append(h_t)
                            _ln(nc, st, consts, x_ts[tt], h_t, "1")
                    fill(3)
                    with tc.high_priority(1200):
                        for tt in range(NT):
                            transpose3(h_ts[tt], hT3, tt * 128, "act")

                    # --- QKV (q,k feature-major; fc order lets hp0 start early)
                    qk = [None] * 6
                    for fc in (0, 3, 1, 4, 2, 5):
                        t = qkp.tile([128, NP], MT, tag="qk", name=f"qk{b}_{fc}")
                        qk[fc] = t
                        for c0, c1 in ((0, 512), (512, NP)):
                            ps = pp.tile([128, 512], F32, tag="fr", name="qk_ps",
                                         space="PSUM", bufs=2)
                            for kc in range(KC_D):
                                nc.tensor.matmul(
                                    ps[:, 0:c1 - c0],
                                    wqkv[:, kc * 3 * D + fc * 128:
                                         kc * 3 * D + fc * 128 + 128],
                                    hT3[:, kc, c0:c1],
                                    start=(kc == 0), stop=(kc == KC_D - 1),
                                )
                            if add_qkb:
                                nc.vector.tensor_scalar(
                                    out=t[:, c0:c1], in0=ps[:, 0:c1 - c0],
                                    scalar1=qkb[:, fc:fc + 1], scalar2=None,
                                    op0=ALU.add)
                            else:
                                nc.vector.tensor_copy(t[:, c0:c1],
                                                      ps[:, 0:c1 - c0])

                    fill(1)
                    # --- v token-major [128, H*VW]: [v | masked-ones | pad]
                    v_sb = []
                    for tt in range(NT):
                        ps = pp.tile([128, 512], F32, tag="fr", name="v_ps",
                                     space="PSUM", bufs=2)
                        for kc in range(KC_D):
                            nc.tensor.matmul(
                                ps[:, 0:D],
                                hT3[:, kc, tt * 128:(tt + 1) * 128],
                                wqkv[:, kc * 3 * D + 768:kc * 3 * D + 1152],
                                start=(kc == 0), stop=(kc == KC_D - 1),
                            )
                        vt = v_ring[(b % 2) * NT + tt]
                        v_sb.append(vt)
                        v3 = vt.rearrange("p (h c) -> p h c", c=VW)
                        ps3 = ps[:, 0:D].rearrange("p (h c) -> p h c", c=HD)
                        if add_vb:
                            nc.vector.tensor_add(
                                v3[:, :, 0:HD], ps3,
                                vbr.rearrange("p (h c) -> p h c", c=HD))
                            if tt == NT - 1:
                                nc.vector.tensor_scalar(
                                    out=v3[:, :, 0:HD], in0=v3[:, :, 0:HD],
                                    scalar1=vone6[:, tt * H * 2:tt * H * 2 + 1],
                                    scalar2=None, op0=ALU.mult)
                        else:
                            # padded-token rows of psum are exactly 0 (h_pad==0)
                            nc.vector.tensor_copy(v3[:, :, 0:HD], ps3)

                    fill(1)
                    # --- scores + poly: all 6 heads (pair-interleaved PE rows)
                    # high priority: the ACT square stretch paces the whole
                    # batch; score matmuls must preempt interleaved MLP work
                    polys = {}
                    for hp in range(H // 2):
                        h0, h1 = 2 * hp, 2 * hp + 1
                        for h in (h0, h1):
                            for mc in range(NT):
                                polys[(h, mc)] = polyp.tile(
                                    [128, NP], MT, tag="poly",
                                    name=f"poly{h}_{mc}")
                                if SIMSAFE:
                                    # pad cols are never consumed (masked via
                                    # v rows); init only for CoreSim checks
                                    nc.vector.memset(
                                        polys[(h, mc)][:, NV:NP], 0.0)
                        prio_ctx = tc.high_priority(1500)
                        prio_ctx.__enter__()
                        for mc in range(NT):
                            pss = {}
                            for h in (h0, h1):
                                fcq = h // 2
                                row = (h % 2) * 64
                                ps = pp.tile([128, 1024], F32, tag="sc",
                                             name="sc_ps", space="PSUM",
                                             bufs=2)
                                pss[h] = ps
                                for c0, c1 in ((0, 512), (512, NV)):
                                    nc.tensor.matmul(
                                        ps[:, c0:c1],
                                        qk[3 + fcq][row:row + 64,
                                                    mc * 128:(mc + 1) * 128],
                                        qk[fcq][row:row + 64, c0:c1],
                                        start=True, stop=True,
                                    )
                            for h in (h0, h1):
                                if polymode == "lin":
                                    nc.scalar.activation(
                                        polys[(h, mc)][:, 0:NV],
                                        pss[h][:, 0:NV], AF.Relu,
                                        scale=pB, bias=c_pc2)
                                elif polymode == "relu":
                                    tsb = tsbp.tile([128, NP], BF16, tag="tsb",
                                                    name="t_sb")
                                    nc.scalar.activation(
                                        tsb[:, 0:NV], pss[h][:, 0:NV],
                                        AF.Square, bias=c_b2a)
                                    nc.scalar.activation(
                                        polys[(h, mc)][:, 0:NV],
                                        tsb[:, 0:NV], AF.Relu,
                                        scale=pA, bias=c_pc2)
                                else:  # fold
                                    tsb = tsbp.tile([128, NP], BF16, tag="tsb",
                                                    name="t_sb")
                                    nc.scalar.activation(
                                        tsb[:, 0:NV], pss[h][:, 0:NV],
                                        AF.Square, bias=c_b2a)
                                    # DVE 4x mode: all-SBUF, 2-byte, packed
                                    nc.vector.tensor_scalar(
                                        out=polys[(h, mc)][:, 0:NV],
                                        in0=tsb[:, 0:NV], scalar1=C2A,
                                        scalar2=0.0, op0=ALU.add, op1=ALU.max)
                        prio_ctx.__exit__(None, None, None)
                        fill(2)

                    # --- attn @ [v | ones]: all heads in one [128,396] psum
                    ao_t = []
                    for nt in range(NT):
                        ps = pp.tile([128, H * VW], F32, tag="mm",
                                     name="av_ps", space="PSUM", bufs=2)
                        for h in range(H):
                            off = h * VW
                            for mc in range(NT):
                                nc.tensor.matmul(
                                    ps[:, off:off + VW],
                                    polys[(h, mc)][:, nt * 128:(nt + 1) * 128],
                                    v_sb[mc][:, off:off + VW],
                                    start=(mc == 0), stop=(mc == NT - 1),
                                )
                        # one copy frees the psum bank; normalize from SBUF
                        avs = st.tile([128, H * VW], F32, tag="avs", name="avs",
                                      bufs=2)
                        nc.vector.tensor_copy(avs, ps)
                        zv = avs.rearrange("p (h c) -> p h c", c=VW)[:, :, HD]
                        zt = st.tile([128, H], F32, tag="zt", name="zt")
                        nc.vector.tensor_scalar_add(zt, zv, float(sc["eps_eff"]))
                        rz = st.tile([128, H], F32, tag="rz", name="rz")
                        nc.vector.reciprocal(rz, zt)
                        ao = aop.tile([128, D], MT, tag="ao", name=f"ao{b}_{nt}")
                        ao_t.append(ao)
                        for h in range(H):
                            eng = nc.vector if h < 3 else nc.gpsimd
                            eng.tensor_scalar(
                                out=ao[:, h * HD:(h + 1) * HD],
                                in0=avs[:, h * VW:h * VW + HD],
                                scalar1=rz[:, h:h + 1], scalar2=None,
                                op0=ALU.mult)

                    # --- transpose attn_out, proj, residual, LN2 (fused) ---
                    aT = aTp.tile([128, KC_D * NP], MT, tag="aT", name=f"aT{b}")
                    aT3 = aT.rearrange("p (k c) -> p k c", c=NP)
                    for nt in range(NT):
                        transpose3(ao_t[nt], aT3, nt * 128, "act")
                    for tt in range(NT):
                        gt = b * NT + tt
                        ps = pp.tile([128, 512], F32, tag="mm", name="pj_ps",
                                     space="PSUM", bufs=2)
                        for kc in range(KC_D):
                            nc.tensor.matmul(
                                ps[:, 0:D],
                                aT3[:, kc, tt * 128:(tt + 1) * 128],
                                wproj[:, kc * D:(kc + 1) * D],
                                start=(kc == 0), stop=(kc == KC_D - 1),
                            )
                        x2t = x2p.tile([128, D], F32, tag="x2", name=f"x2_{gt}")
                        x2_tiles[gt] = x2t
                        if add_pb:
                            nc.vector.tensor_add(x2t, ps[:, 0:D], pbr)
                            nc.vector.tensor_add(x2t, x2t, x_ts[tt])
                        else:
                            nc.vector.tensor_add(x2t, ps[:, 0:D], x_ts[tt])
                        # LN2 per tile right after its residual: the h2T
                        # transpose is in flight while the next proj runs
                        h2_t = hpool.tile([128, D], MT, tag="h2", name="h2_t")
                        _ln(nc, st, consts, x2t, h2_t, "2")
                        transpose3(h2_t, h2T3, gt * 128, "act")

                def mlp_chunk(nch):
                    """Generator: yields after each piece so the caller can
                    interleave MLP work into the next batch's attention."""
                    g_sb = []
                    for fc in range(FC_H):
                        ps = pp.tile([128, 512], F32, tag="mm", name="f1_ps",
                                     space="PSUM", bufs=2)
                        for kc in range(KC_D):
                            nc.tensor.matmul(
                                ps,
                                wfc1[:, kc * HID + fc * 128:
                                     kc * HID + fc * 128 + 128],
                                h2T3[:, kc, nch * 512:(nch + 1) * 512],
                                start=(kc == 0), stop=(kc == KC_D - 1),
                            )
                        gt_sb = gp.tile([128, 512], MT, tag="g",
                                        name=f"g{nch}_{fc}")
                        g_sb.append(gt_sb)
                        nc.scalar.activation(
                            gt_sb, ps,
                            AF.Square if gelmode == "quad" else AF.Identity,
                            bias=g1b[:, fc:fc + 1])
                        if fc % 3 == 2:
                            yield
                    ota = outpl.tile([128, 4 * D], F32, tag="ot", name="out_a")
                    ota3 = ota.rearrange("p (t c) -> p t c", c=D)
                    for tt in range(4):
                        gt = nch * 4 + tt
                        ps = pp.tile([128, 512], F32, tag="mm", name="f2_ps",
                                     space="PSUM", bufs=2)
                        for fc in range(FC_H):
                            nc.tensor.matmul(
                                ps[:, 0:D],
                                g_sb[fc][:, tt * 128:(tt + 1) * 128],
                                wfc2[:, fc * D:(fc + 1) * D],
                                start=(fc == 0), stop=(fc == FC_H - 1),
                            )
                        ot = ota3[:, tt, :]
                        if add_f2b:
                            nc.vector.tensor_add(ot, ps[:, 0:D], f2br)
                            nc.vector.tensor_add(ot, ot, x2_tiles[gt])
                        else:
                            nc.vector.tensor_add(ot, ps[:, 0:D], x2_tiles[gt])
                        yield
                    # out DMA on the Pool SWDGE queue: SP head-of-line
                    # blocking would park x-prefetch/h2T behind this wait
                    if bench_R:
                        nc.gpsimd.dma_start(outp[0:128, :], ota3[:, 0, :])
                    else:
                        dst_v = outp[nch * 512:(nch + 1) * 512, :].rearrange(
                            "(t p) c -> p t c", p=128)
                        nc.gpsimd.dma_start(dst_v, ota3)

                prefetch_x(0)
                _late_const_dmas()
                _init_v_ring()
                pending = []  # MLP piece generators ready for interleaving

                def fill(n):
                    for _ in range(n):
                        while pending:
                            if next(pending[0], StopIteration) is StopIteration:
                                pending.pop(0)
                                continue
                            break
                        else:
                            return

                for b in range(BPC):
                    attention(b, fill)
                    fill(10 ** 6)  # drain leftovers
                    ready = ((b + 1) * NP) // 512
                    while mlp_done[0] < ready:
                        pending.append(mlp_chunk(mlp_done[0]))
                        mlp_done[0] += 1
                fill(10 ** 6)

        if bench_R:
            with tc.For_i(0, bench_R, 1):
                _body_phases()
        else:
            _body_phases()

    nc.compile()
    return nc


def host_prep(inputs):
    """Fold LN affine params into weights, build per-core input maps."""
    f = lambda k: np.asarray(inputs[k], dtype=np.float32)
    x = f("x")
    qkv_w, qkv_b = f("qkv_w"), f("qkv_b")
    proj_w, proj_b = f("proj_w"), f("proj_b")
    fc1_w, fc1_b = f("fc1_w"), f("fc1_b")
    fc2_w, fc2_b = f("fc2_w"), f("fc2_b")
    ln1_g, ln1_b = f("ln1_g"), f("ln1_b")
    ln2_g, ln2_b = f("ln2_g"), f("ln2_b")
    attn_abc = f("attn_abc")
    gelu_abc = f("gelu_abc")

    scale = HD ** -0.5
    pA = float(attn_abc[0]) * scale * scale
    pB = float(attn_abc[1]) * scale
    pC = float(attn_abc[2])
    sc = {"pA": pA, "pB": pB, "pC": pC}
    if pA > 1e-12:
        sc["polymode"] = "fold"
        sc["B2A"] = pB / (2.0 * pA)
        sc["pC2"] = pC - pB * pB / (4.0 * pA)
        sc["C2A"] = sc["pC2"] / pA
        eps_eff = ATTN_EPS / pA
    elif pA < -1e-12:
        sc["polymode"] = "relu"
        sc["B2A"] = pB / (2.0 * pA)
        sc["pC2"] = pC - pB * pB / (4.0 * pA)
        sc["C2A"] = 0.0
        eps_eff = ATTN_EPS
    else:
        sc["polymode"] = "lin"
        sc["B2A"] = 0.0
        sc["pC2"] = pC
        sc["C2A"] = 0.0
        eps_eff = ATTN_EPS

    ga, gb, gc = float(gelu_abc[0]), float(gelu_abc[1]), float(gelu_abc[2])
    fc1_b_eff = (fc1_b + ln2_b @ fc1_w).astype(np.float32)
    if abs(ga) > 1e-12:
        # gelu(u) = ga*(u + gb/2ga)^2 + (gc - gb^2/4ga); fold ga into fc2_w
        # and the constant into fc2_b via column sums.
        sc["gelmode"] = "quad"
        gC2 = gc - gb * gb / (4.0 * ga)
        g1bias = fc1_b_eff + gb / (2.0 * ga)
        fc2_w_eff = ga * fc2_w
        fc2_b_eff = fc2_b + gC2 * fc2_w.sum(axis=0)
    else:
        # gelu(u) = gb*u + gc
        sc["gelmode"] = "lin"
        g1bias = fc1_b_eff
        fc2_w_eff = gb * fc2_w
        fc2_b_eff = fc2_b + gc * fc2_w.sum(axis=0)

    sc["eps_eff"] = float(eps_eff)
    qkv_w_eff = (ln1_g[:, None] * qkv_w).astype(np.float32)
    qkv_b_eff = (qkv_b + ln1_b @ qkv_w).astype(np.float32)
    fc1_w_eff = (ln2_g[:, None] * fc1_w).astype(np.float32)

    sc["add_vb"] = bool(np.any(qkv_b_eff[2 * D:] != 0.0))
    sc["add_pb"] = bool(np.any(proj_b != 0.0))
    sc["add_f2b"] = bool(np.any(fc2_b_eff != 0.0))
    sc["add_qkb"] = bool(np.any(qkv_b_eff[:2 * D] != 0.0))

    common = {
        "wqkv": np.ascontiguousarray(
            qkv_w_eff.reshape(KC_D, 128, 3 * D).transpose(1, 0, 2)
            .reshape(128, KC_D * 3 * D)).astype(NPBF),
        "wproj": np.ascontiguousarray(
            proj_w.reshape(KC_D, 128, D).transpose(1, 0, 2)
            .reshape(128, KC_D * D)).astype(NPBF),
        "wfc1": np.ascontiguousarray(
            fc1_w_eff.reshape(KC_D, 128, HID).transpose(1, 0, 2)
            .reshape(128, KC_D * HID)).astype(NPBF),
        "wfc2": np.ascontiguousarray(
            fc2_w_eff.astype(np.float32).reshape(FC_H, 128, D).transpose(1, 0, 2)
            .reshape(128, FC_H * D)).astype(NPBF),
        "qkb": np.ascontiguousarray(qkv_b_eff[:2 * D].reshape(6, 128).T),
        "vbr": np.ascontiguousarray(
            np.broadcast_to(qkv_b_eff[2 * D:], (128, D))),
        "pbr": np.ascontiguousarray(np.broadcast_to(proj_b, (128, D))),
        "f2br": np.ascontiguousarray(np.broadcast_to(fc2_b_eff, (128, D))),
        "g1b": np.ascontiguousarray(g1bias.reshape(FC_H, 128).T),
        "vone6": None,
        "ident": np.eye(128, dtype=np.float32).astype(NPBF),
    }
    mask = (np.arange(NP) < N).astype(np.float32)  # [640]
    mz = np.zeros((NT, 128, H, 2), np.float32)
    mz[:, :, :, 0] = mask.reshape(NT, 128)[:, :, None]
    common["vone6"] = np.ascontiguousarray(
        mz.transpose(1, 0, 2, 3).reshape(128, NT * H * 2))
    common = {k: (np.ascontiguousarray(v, dtype=np.float32)
                  if v.dtype != NPBF else v)
              for k, v in common.items()}

    in_maps = []
    for c in range(NCORES):
        xp_c = np.zeros((BPC, NP, D), np.float32)
        xp_c[:, :N, :] = x[c * BPC:(c + 1) * BPC]
        m = dict(common)
        m["xp"] = xp_c.reshape(TP, D)
        in_maps.append(m)
    return sc, in_maps


_CACHE = {}


def _get_program(sc):
    key = tuple(sorted((k, v) for k, v in sc.items()))
    if key not in _CACHE:
        _CACHE[key] = build_program(sc)
    return _CACHE[key]


def _runner_meta(nc):
    partition_name = nc.partition_id_tensor.name if nc.partition_id_tensor else None
    in_names, out_names, out_avals, zero_outs = [], [], [], []
    import jax
    for alloc in nc.m.functions[0].allocations:
        if not isinstance(alloc, mybir.MemoryLocationSet):
            continue
        name = alloc.memorylocations[0].name
        if alloc.kind == "ExternalInput":
            if name != partition_name:
                in_names.append(name)
        elif alloc.kind == "ExternalOutput":
            out_names.append(name)
            shape = tuple(alloc.tensor_shape)
            dtype = mybir.dt.np(alloc.dtype)
            out_avals.append(jax.core.ShapedArray(shape, dtype))
            zero_outs.append(np.zeros(shape, dtype))
    return partition_name, in_names, out_names, out_avals, zero_outs


_RUNNERS = {}


def _make_runner(nc, chain):
    """Jitted 8-core runner executing the NEFF `chain` times back-to-back
    (iteration i+1 consumes iteration i's outputs as its scratch buffers,
    forcing sequential execution)."""
    key = (id(nc), chain)
    if key in _RUNNERS:
        return _RUNNERS[key]
    import jax
    from jax.sharding import Mesh, PartitionSpec, NamedSharding
    from jax.experimental.shard_map import shard_map
    from concourse.bass2jax import (_bass_exec_p, install_neuronx_cc_hook,
                                    partition_id_tensor)
    install_neuronx_cc_hook()
    partition_name, in_names, out_names, out_avals, zero_outs = _runner_meta(nc)
    n_params = len(in_names)
    all_in = list(in_names) + list(out_names)
    if partition_name is not None:
        all_in = all_in + [partition_name]

    def _body(*args):
        ins = list(args[:n_params])
        cur = list(args[n_params:])
        for _ in range(chain):
            operands = ins + cur
            if partition_name is not None:
                operands = operands + [partition_id_tensor()]
            cur = list(_bass_exec_p.bind(
                *operands,
                out_avals=tuple(out_avals),
                in_names=tuple(all_in),
                out_names=tuple(out_names),
                lowering_input_output_aliases=(),
                sim_require_finite=True,
                sim_require_nnan=True,
                nc=nc,
            ))
        return tuple(cur)

    devices = jax.devices()[:NCORES]
    mesh = Mesh(np.asarray(devices), ("core",))
    nin = n_params + len(out_names)
    sharded = jax.jit(
        shard_map(_body, mesh=mesh,
                  in_specs=(PartitionSpec("core"),) * nin,
                  out_specs=(PartitionSpec("core"),) * len(out_names),
                  check_rep=False),
        keep_unused=True)
    shard = NamedSharding(mesh, PartitionSpec("core"))
    r = (sharded, shard, in_names, out_names, zero_outs)
    _RUNNERS[key] = r
    return r


def _concat_inputs(in_maps, in_names, zero_outs):
    concat_in = [np.concatenate([np.asarray(in_maps[c][n]) for c in range(NCORES)],
                                axis=0) for n in in_names]
    concat_zero = [np.zeros((NCORES * z.shape[0], *z.shape[1:]), z.dtype)
                   for z in zero_outs]
    return concat_in, concat_zero


def kernel(**inputs):
    sc, in_maps = host_prep(inputs)
    nc = _get_program(sc)
    sharded, shard, in_names, out_names, zero_outs = _make_runner(nc, 1)
    concat_in, concat_zero = _concat_inputs(in_maps, in_names, zero_outs)
    out_arrs = sharded(*concat_in, *concat_zero)
    oi = out_names.index("outp")
    full = np.asarray(out_arrs[oi]).reshape(NCORES, BPC, NP, D)[:, :, :N, :]
    return np.ascontiguousarray(full.reshape(B, N, D), dtype=np.float32)


def bench(inputs, chain=65, reps=15):
    """Measure per-execution HW time: bench-variant programs with internal
    (unfed) inputs and an in-program For_i repeat loop; difference R=chain
    vs R=1 wall time to cancel dispatch/transfer overhead."""
    import time
    import jax
    sc, in_maps = host_prep(inputs)

    def _run_R(R):
        key = (tuple(sorted((k, v) for k, v in sc.items())), "bench", R)
        if key not in _CACHE:
            _CACHE[key] = build_program(sc, bench_R=R)
        nc = _CACHE[key]
        sharded, shard, in_names, out_names, zero_outs = _make_runner(nc, 1)
        concat_in, concat_zero = _concat_inputs(
            [dict() for _ in range(NCORES)], in_names, zero_outs)
        out = sharded(*concat_in, *concat_zero)
        jax.block_until_ready(out)
        ts = []
        for _ in range(reps):
            t0 = time.perf_counter()
            out = sharded(*concat_in, *concat_zero)
            jax.block_until_ready(out)
            ts.append(time.perf_counter() - t0)
        return min(ts)

    t1 = _run_R(1)
    tn = _run_R(chain)
    per_exec_ns = (tn - t1) / (chain - 1) * 1e9
    return per_exec_ns, t1, tn
